# revision 87
# baseline (speedup 1.0000x reference)
"""Trainium2 Bass kernel for a 3-layer transformer encoder (self+cross attention, FFN).

Data-parallel over batch: 64 batches split as 8 per NeuronCore.
Residual stream kept feature-major fp32 [DIM partitions, tokens].

Precision/perf strategy:
- All attention projections (Q/K/V/O, self+cross) run as fp8-e4m3 DoubleRow
  matmuls (two k-tiles contracted per instruction at 0.5 cycles/row), with
  power-of-2 per-tensor weight scales computed on the host and descale factors
  folded into existing activation copies / the softmax exp / the residual add.
- The FFN stays bf16 (fp8 there costs ~2e-2 rel err); its weights are staged
  once per layer in a contiguous per-partition dram layout, with hT holding
  both 512-token chunks so W2 streams only once.
- Attention computes sim transposed ([k, q] per head) so the fp8 exp output
  feeds attn@v directly (no transposes / psum round trips); softmax
  denominators come from an all-ones-stationary PE matmul broadcast across
  partitions, and the normalization is a reciprocal-multiply on the [dh, q]
  attention output. The out-projection is interleaved into the attention batch
  loop in 2-m-tile pieces to fill PE bubbles.
- RoPE: rotate-half is one PE matmul against a constant shift matrix; the two
  elementwise multiplies are split across Pool (cos, all-SBUF bf16) and DVE
  (sin, psum operand); rmsnorm statistics use fp8-DR matmuls and a
  recip+Sqrt tail (avoids the Ln/Exp act-table reload thrash).
"""

import numpy as np
import ml_dtypes

import concourse.mybir as mybir
import concourse.tile as tile
from concourse import bacc
from concourse.bass_utils import run_bass_kernel_spmd
from concourse.masks import make_identity

BF16 = ml_dtypes.bfloat16
E4 = ml_dtypes.float8_e4m3
F32 = mybir.dt.float32
BF = mybir.dt.bfloat16
F8 = mybir.dt.float8e4
DRMODE = mybir.MatmulPerfMode.DoubleRow
AF = mybir.ActivationFunctionType
ALU = mybir.AluOpType
AX = mybir.AxisListType

SXA = 32.0   # fp8 scale for rmsnormed activations
SXC = 32.0   # fp8 scale for raw chunked_seq
SO = 16.0    # fp8 scale for attention output (pre out-projection)
S8Q = 256.0  # fp8 scale for rope'd q
S8K = 16.0   # fp8 scale for rope'd k

DIM = 1024
HEADS = 16
DH = 64
INNER = 1024
FF = 4096
ROT = 32
N_CTX = 128   # query tokens per batch
M_CTX = 64    # chunk tokens per batch
N_CORES = 8
KT = DIM // 128  # 8 k-subtiles for DIM contraction


def _build(BB, DEPTH, scales, gelu_exact=True, has_bias=False):
    """Build + compile the per-core bass program for BB local batches, DEPTH layers."""
    TOK = BB * N_CTX
    TOKC = BB * M_CTX
    CH = min(512, TOK)
    NCH = TOK // CH
    CHC = min(512, TOKC)

    swq, swk, swv, swo, swqc, swkc, swvc, swoc, gout_ones = scales
    # rope outputs are rescaled into fp8 range via the STT scalar:
    cq_a = S8Q / (SXA * swq)
    ck_a = S8K / (SXA * swk)
    cq_c = S8Q / (SXA * swqc)
    ck_c = S8K / (SXC * swkc)
    exp_s = 1.0 / (S8Q * S8K)    # same for self and cross
    vdesc_a = SO / (SXA * swv)   # vv carries x16 so o_all = e@vv/d lands at xSO
    vdesc_c = SO / (SXC * swvc)
    odesc_a = 1.0 / (SO * swo)
    odesc_c = 1.0 / (SO * swoc)

    nc = bacc.Bacc("TRN2", target_bir_lowering=False, debug=False)

    xT_in = nc.dram_tensor("xT", [DIM, TOK], F32, kind="ExternalInput")
    csT_in = nc.dram_tensor("csT", [DIM, TOKC], F8, kind="ExternalInput")
    # staged-weight dram layout: [..., block, 128 partitions, KT, 512] so each
    # staging DMA reads one contiguous row per partition (128 descriptors).
    wq = nc.dram_tensor("wq", [DEPTH, 2, 128, KT, 512], F8, kind="ExternalInput")
    wk = nc.dram_tensor("wk", [DEPTH, 2, 128, KT, 512], F8, kind="ExternalInput")
    wv = nc.dram_tensor("wv", [DEPTH, 2, 128, KT, 512], F8, kind="ExternalInput")
    wo = nc.dram_tensor("wo", [DEPTH, 2, 128, KT, 512], F8, kind="ExternalInput")
    wqc = nc.dram_tensor("wqc", [DEPTH, 2, 128, KT, 512], F8, kind="ExternalInput")
    wkc = nc.dram_tensor("wkc", [DEPTH, 2, 128, KT, 512], F8, kind="ExternalInput")
    wvc = nc.dram_tensor("wvc", [DEPTH, 2, 128, KT, 512], F8, kind="ExternalInput")
    woc = nc.dram_tensor("woc", [DEPTH, 2, 128, KT, 512], F8, kind="ExternalInput")
    w1 = nc.dram_tensor("w1", [DEPTH, 8, 128, KT, 512], BF, kind="ExternalInput")
    w2 = nc.dram_tensor("w2", [DEPTH, 4, 4, 128, KT, 256], BF, kind="ExternalInput")
    b1c = nc.dram_tensor("b1c", [DEPTH, 128, FF // 128], F32, kind="ExternalInput")
    goutc = nc.dram_tensor("goutc", [128, KT], F32, kind="ExternalInput")
    shiftm_in = nc.dram_tensor("shiftm", [128, 128], BF, kind="ExternalInput")
    if has_bias:
        brows = nc.dram_tensor("brows", [DEPTH, 3, DIM], BF, kind="ExternalInput")
    cosq_in = nc.dram_tensor("cosq", [128, CH], BF, kind="ExternalInput")
    sinq_in = nc.dram_tensor("sinq", [128, CH], BF, kind="ExternalInput")
    cosc_in = nc.dram_tensor("cosc", [128, CHC], BF, kind="ExternalInput")
    sinc_in = nc.dram_tensor("sinc", [128, CHC], BF, kind="ExternalInput")

    outT = nc.dram_tensor("outT", [DIM, TOK], F32, kind="ExternalOutput")

    with tile.TileContext(nc) as tc:
        with tc.tile_pool(name="singles", bufs=1) as singles, \
             tc.tile_pool(name="big", bufs=1) as big, \
             tc.tile_pool(name="wst", bufs=3) as wst, \
             tc.tile_pool(name="scratch", bufs=2) as scratch, \
             tc.tile_pool(name="qrawp", bufs=1) as qrawp, \
             tc.tile_pool(name="attn", bufs=2) as attn_pool, \
             tc.tile_pool(name="psmm", bufs=4, space="PSUM") as psmm, \
             tc.tile_pool(name="psat", bufs=1, space="PSUM") as psat, \
             tc.tile_pool(name="psnrm", bufs=1, space="PSUM") as psnrm:

            # ---------- constants ----------
            ones_col = singles.tile([128, 1], BF)
            nc.vector.memset(ones_col, 1.0)
            ones8w = singles.tile([128, 2, 64], F8)
            nc.vector.memset(ones8w, 1.0)
            ones_mat = singles.tile([128, 128], BF)
            nc.vector.memset(ones_mat, 1.0)
            ones_row_b = singles.tile([1, CH], BF)
            nc.vector.memset(ones_row_b, 1.0)
            cosq = singles.tile([128, CH], BF)
            nc.sync.dma_start(out=cosq, in_=cosq_in[:])
            sinq = singles.tile([128, CH], BF)
            nc.sync.dma_start(out=sinq, in_=sinq_in[:])
            cosc = singles.tile([128, CHC], BF)
            nc.sync.dma_start(out=cosc, in_=cosc_in[:])
            sinc = singles.tile([128, CHC], BF)
            nc.sync.dma_start(out=sinc, in_=sinc_in[:])
            b1_sb = singles.tile([128, DEPTH, FF // 128], F32)
            nc.sync.dma_start(out=b1_sb, in_=b1c.rearrange("l p m -> p l m"))
            gout_sb = singles.tile([128, KT], F32)
            nc.sync.dma_start(out=gout_sb, in_=goutc[:])
            shiftm = singles.tile([128, 128], BF)
            nc.sync.dma_start(out=shiftm, in_=shiftm_in[:])
            if has_bias:
                brows_sb = singles.tile([1, DEPTH, 3, DIM], BF)
                nc.sync.dma_start(out=brows_sb, in_=brows[None])

            xT = singles.tile([128, KT, TOK], F32)
            in_engines = [nc.sync, nc.scalar, nc.gpsimd]
            for k in range(KT):
                in_engines[k % 3].dma_start(
                    out=xT[:, k],
                    in_=xT_in.rearrange("(kt p) t -> p kt t", p=128)[:, k])
            csT = singles.tile([128, KT, TOKC], F8)
            nc.scalar.dma_start(out=csT, in_=csT_in.rearrange("(kt p) t -> p kt t", p=128))

            # ---------- rmsnorm ----------
            def rmsnorm(dst, dst_f32=None, gcol=None, sx=1.0):
                sq = big.tile([128, KT, TOK], F8, tag="obuf", name="sq")  # obuf slot is dead here
                for c in range(NCH):
                    sl = slice(c * CH, (c + 1) * CH)
                    for k in range(KT):
                        if k % 3 == 0:
                            nc.scalar.activation(out=sq[:, k, sl], in_=xT[:, k, sl],
                                                 func=AF.Square)
                        elif k % 3 == 1:
                            nc.vector.tensor_tensor(out=sq[:, k, sl], in0=xT[:, k, sl],
                                                    in1=xT[:, k, sl], op=ALU.mult)
                        else:
                            nc.gpsimd.tensor_tensor(out=sq[:, k, sl], in0=xT[:, k, sl],
                                                    in1=xT[:, k, sl], op=ALU.mult)
                    sp = psnrm.tile([64, CH], F32, tag="nrm", name="nrm")
                    for k in range(0, KT, 2):
                        nc.tensor.matmul(sp, lhsT=ones8w, rhs=sq[:, k:k + 2, sl],
                                         start=(k == 0), stop=(k == KT - 2),
                                         perf_mode=DRMODE)
                    # rstd = sx/sqrt(ms) via DVE recip + ACT Sqrt (Sqrt/Copy/
                    # Square share act tables with everything, unlike Ln/Exp
                    # whose alternation forced a table reload per chunk)
                    rr = scratch.tile([1, CH], BF, tag="rstd", name="rr")
                    with nc.allow_low_precision(reason="bf16 rmsnorm recip"):
                        nc.vector.reciprocal(out=rr, in_=sp[0:1])
                    rstd_b = scratch.tile([1, CH], BF, tag="rstdb", name="rstd_b")
                    nc.scalar.activation(out=rstd_b, in_=rr, func=AF.Sqrt,
                                         scale=DIM * sx * sx, bias=0.0)
                    bp = psnrm.tile([128, CH], F32, tag="nrm", name="nrm")
                    nc.tensor.matmul(bp, lhsT=ones_row_b[:, :128], rhs=rstd_b,
                                     start=True, stop=True)
                    bp_sb = scratch.tile([128, CH], BF, tag="bpsb", name="bp_sb")
                    nc.scalar.copy(out=bp_sb, in_=bp)
                    for k in range(KT):
                        if dst is not None:
                            eng = nc.vector if k % 2 == 0 else nc.gpsimd
                            eng.tensor_tensor(out=dst[:, k, sl], in0=xT[:, k, sl],
                                              in1=bp_sb, op=ALU.mult)
                        if dst_f32 is not None:
                            of = scratch.tile([128, CH], F32, tag="outf", name="outf")
                            if gcol is None:  # g_out == 1: plain normalize
                                eng2 = nc.vector if k % 2 == 0 else nc.gpsimd
                                eng2.tensor_tensor(out=of, in0=xT[:, k, sl],
                                                   in1=bp_sb, op=ALU.mult)
                            else:
                                nc.vector.scalar_tensor_tensor(
                                    out=of, in0=xT[:, k, sl], scalar=gcol[:, k, None],
                                    in1=bp_sb, op0=ALU.mult, op1=ALU.mult)
                            eng = nc.sync if k % 2 == 0 else nc.scalar
                            eng.dma_start(out=dst_f32[k * 128:(k + 1) * 128, sl],
                                          in_=of)

            # ---------- rope-packed q/k projection (fp8 DoubleRow) ----------
            def qk_project(dst, wdram, layer, src, n_src_tok, cos_t, sin_t, crs):
                ch = min(512, n_src_tok)
                nch = n_src_tok // ch
                for sg in range(2):  # m-tiles 4*sg .. 4*sg+3
                    wn = wst.tile([128, KT, 512], F8, tag="wstage", name="wstage")
                    nc.sync.dma_start(out=wn, in_=wdram[layer, sg])
                    for c in range(nch):
                        sl = slice(c * ch, (c + 1) * ch)
                        qps, qraws = [], []
                        for m in range(4):
                            qp = psmm.tile([128, 512], F32, tag="mm", name="mm")[:, :ch]
                            for k in range(0, KT, 2):
                                nc.tensor.matmul(qp, lhsT=wn[:, k:k + 2, m * 128:(m + 1) * 128],
                                                 rhs=src[:, k:k + 2, sl],
                                                 start=(k == 0), stop=(k == KT - 2),
                                                 perf_mode=DRMODE)
                            # the fp8-range rescale rides the psum->sbuf copy
                            qraw = qrawp.tile([128, 512], BF, tag=f"qraw{m}",
                                                name=f"qraw{m}")[:, :ch]
                            nc.scalar.activation(out=qraw, in_=qp, func=AF.Copy,
                                                 scale=crs)
                            qps.append(qp); qraws.append(qraw)
                        for m in range(4):
                            rp = psat.tile([128, 512], F32, tag="op", name="op")[:, :ch]
                            nc.tensor.matmul(rp, lhsT=shiftm, rhs=qraws[m],
                                             start=True, stop=True)
                            # qraw·cos on Pool (all-SBUF), rp·sin on DVE
                            tcos = scratch.tile([128, 512], BF, tag="tcos", name="tcos")[:, :ch]
                            nc.gpsimd.tensor_tensor(out=tcos, in0=qraws[m],
                                                    in1=cos_t[:, :ch], op=ALU.mult)
                            tsin = scratch.tile([128, 512], BF, tag="tsin", name="tsin")[:, :ch]
                            nc.vector.tensor_tensor(out=tsin, in0=rp,
                                                    in1=sin_t[:, :ch], op=ALU.mult)
                            nc.gpsimd.tensor_tensor(out=dst[:, 4 * sg + m, sl],
                                                    in0=tcos, in1=tsin, op=ALU.add)

            # ---------- v projection (token-major, fp8 DoubleRow) ----------
            def v_project(dst, wdram, layer, src, n_src_tok, cross, vdesc):
                wsb = []
                for g in range(2):
                    wt = wst.tile([128, KT, 512], F8, tag="wstage", name="wstage")
                    nc.sync.dma_start(out=wt, in_=wdram[layer, g])
                    wsb.append(wt)
                for mt in range(n_src_tok // 128):
                    for g in range(2):
                        vp = psmm.tile([128, 512], F32, tag="mm", name="mm")
                        for k in range(0, KT, 2):
                            nc.tensor.matmul(vp, lhsT=src[:, k:k + 2, mt * 128:(mt + 1) * 128],
                                             rhs=wsb[g][:, k:k + 2, :],
                                             start=(k == 0), stop=(k == KT - 2),
                                             perf_mode=DRMODE)
                        if not cross:
                            nc.scalar.activation(out=dst[:, mt, g * 512:(g + 1) * 512],
                                                 in_=vp, func=AF.Copy, scale=vdesc)
                        else:
                            nc.scalar.activation(out=dst[0:64, 2 * mt, g * 512:(g + 1) * 512],
                                                 in_=vp[0:64], func=AF.Copy, scale=vdesc)
                            nc.scalar.activation(out=dst[64:128, 2 * mt + 1, g * 512:(g + 1) * 512],
                                                 in_=vp[64:128], func=AF.Copy, scale=vdesc)

            # ---------- attention core ([k,q]-layout, fp8 e, PE denominators) ----------
            def attention(qT, kT, vv, o_all, n_k, cross, tail=None):
                # sim is computed transposed (out [k, q] per head) so the exp
                # output feeds attn@v directly; softmax denominators come from
                # a ones-matmul over the k partitions; the divide happens on
                # the [dh, q] attention output (16x fewer elements than e).
                order = [2 * i for i in range(HEADS // 2)] + [2 * i + 1 for i in range(HEADS // 2)]
                pos = {h: i for i, h in enumerate(order)}

                def phase_a(b):
                    base = 64 * (b % 2) if cross else 0
                    qsl = slice(b * 128, (b + 1) * 128)
                    ksl = slice(b * n_k, (b + 1) * n_k)
                    e_b = attn_pool.tile([128, HEADS * 128], F8, tag="e_b", name="e_b")
                    for g in range(4):  # 4 heads per psum tile
                        heads = order[g * 4:(g + 1) * 4]
                        sp = psmm.tile([128, 512], F32, tag="mm", name="mm")
                        for j, h in enumerate(heads):
                            hb = 64 * (h % 2)
                            nc.tensor.matmul(
                                sp[base:base + n_k, j * 128:(j + 1) * 128],
                                lhsT=kT[hb:hb + 64, h // 2, ksl],
                                rhs=qT[hb:hb + 64, h // 2, qsl],
                                start=(j == 0), stop=(j == 3),
                                skip_group_check=True)
                        nc.scalar.activation(out=e_b[base:base + n_k, g * 512:(g + 1) * 512],
                                             in_=sp[base:base + n_k], func=AF.Exp,
                                             scale=exp_s)
                    return e_b

                def phase_b(b, e_b):
                    base = 64 * (b % 2) if cross else 0
                    qsl = slice(b * 128, (b + 1) * 128)
                    for half in range(2):
                        # denominators for this half's 8 heads, broadcast to all
                        # 64 partitions by an all-ones stationary: evens on top
                        # partitions, odds on bottom (matches op_ layout)
                        dD = psat.tile([128, 512], F32, tag="op", name="dD")
                        for par in range(2):
                            esl = slice(par * 1024 + half * 512,
                                        par * 1024 + half * 512 + 512)
                            nc.tensor.matmul(dD[64 * par:64 * par + 64],
                                             lhsT=ones_mat[base:base + n_k, :64],
                                             rhs=e_b[base:base + n_k, esl],
                                             start=True, stop=True,
                                             skip_group_check=True)
                        dr = attn_pool.tile([128, 512], BF, tag="dr", name="dr")
                        with nc.allow_low_precision(reason="bf16 softmax denom"):
                            nc.vector.reciprocal(out=dr, in_=dD)
                        op_ = psat.tile([128, 512], F32, tag="op2", name="op",
                                        bufs=2)
                        for hp in range(4):
                            h0 = 8 * half + 2 * hp
                            nc.tensor.matmul(
                                op_[0:64, hp * 128:(hp + 1) * 128],
                                lhsT=vv[base:base + n_k, b, h0 * 64:(h0 + 1) * 64],
                                rhs=e_b[base:base + n_k, pos[h0] * 128:(pos[h0] + 1) * 128],
                                start=(hp == 0), stop=False, tile_position=(base, 0),
                                skip_group_check=True)
                            nc.tensor.matmul(
                                op_[64:128, hp * 128:(hp + 1) * 128],
                                lhsT=vv[base:base + n_k, b, (h0 + 1) * 64:(h0 + 2) * 64],
                                rhs=e_b[base:base + n_k, pos[h0 + 1] * 128:(pos[h0 + 1] + 1) * 128],
                                start=(hp == 0), stop=(hp == 3), tile_position=(base, 64),
                                skip_group_check=True)
                        nc.vector.tensor_tensor(
                            out=o_all[:, 4 * half:4 * half + 4, qsl],
                            in0=op_.rearrange("p (hp t) -> p hp t", hp=4),
                            in1=dr.rearrange("p (hp t) -> p hp t", hp=4),
                            op=ALU.mult)

                prev = None
                for b in range(BB):
                    e_b = phase_a(b)
                    if prev is not None:
                        phase_b(prev[0], prev[1])
                        # once batches 0..3 (= token chunk 0) are done, spread
                        # the chunk-0 out-projection pieces over the remaining
                        # batch iterations to fill PE bubbles
                        if prev[0] >= 3 and tail is not None:
                            tail(0, prev[0] - 3)
                    prev = (b, e_b)
                phase_b(prev[0], prev[1])
                if tail is not None:
                    for piece in range(4):
                        tail(1, piece)

            # ---------- output projection + residual (fp8 DoubleRow) ----------
            def out_project_staged(wdram, layer):
                wsb = []
                for g in range(2):
                    wt = wst.tile([128, KT, 512], F8, tag="wstage", name="wstage")
                    nc.sync.dma_start(out=wt, in_=wdram[layer, g])
                    wsb.append(wt)
                return wsb

            def out_project_chunk(wsb, src, odesc, c, piece):
                sl = slice(c * CH, (c + 1) * CH)
                for m in range(2 * piece, 2 * piece + 2):
                    pp = psmm.tile([128, 512], F32, tag="mm", name="mm")[:, :CH]
                    for k in range(0, KT, 2):
                        nc.tensor.matmul(pp,
                                         lhsT=wsb[m // 4][:, k:k + 2, (m % 4) * 128:(m % 4 + 1) * 128],
                                         rhs=src[:, k:k + 2, sl],
                                         start=(k == 0), stop=(k == KT - 2),
                                         perf_mode=DRMODE)
                    nc.vector.scalar_tensor_tensor(out=xT[:, m, sl], in0=pp,
                                                   scalar=odesc, in1=xT[:, m, sl],
                                                   op0=ALU.mult, op1=ALU.add)

            # ---------- FFN (weights staged once; hT holds both chunks) ----------
            def ffn(layer, xn):
                gelu_f = AF.Gelu if gelu_exact else AF.Square
                hT = big.tile([128, FF // 128, TOK], BF, tag="obuf", name="obuf")
                for g in range(FF // 512):
                    wt = wst.tile([128, KT, 512], BF, tag="wstage", name="wstage")
                    nc.sync.dma_start(out=wt, in_=w1[layer, g])
                    for c in range(NCH):
                        sl = slice(c * CH, (c + 1) * CH)
                        for mm in range(4):
                            fm = 4 * g + mm
                            hp = psmm.tile([128, 512], F32, tag="mm", name="mm")[:, :CH]
                            for k in range(KT):
                                nc.tensor.matmul(hp, lhsT=wt[:, k, mm * 128:(mm + 1) * 128],
                                                 rhs=xn[:, k, sl],
                                                 start=(k == 0), stop=(k == KT - 1))
                            nc.scalar.activation(out=hT[:, fm, sl], in_=hp, func=gelu_f,
                                                 bias=b1_sb[:, layer, fm, None], scale=1.0)
                for qm in range(4):  # 256-wide output blocks, both chunks at once
                    yps = [psmm.tile([128, 512], F32, tag="mm", name="mm")[:, :CH]
                           for _ in range(4)]  # index 2*c + mm
                    for kg in range(4):
                        wt = wst.tile([128, KT, 256], BF, tag="wstage2", name="wstage2")
                        nc.sync.dma_start(out=wt, in_=w2[layer, kg, qm])
                        for c in range(NCH):
                            sl = slice(c * CH, (c + 1) * CH)
                            for k in range(KT):
                                for mm in range(2):
                                    nc.tensor.matmul(
                                        yps[2 * c + mm],
                                        lhsT=wt[:, k, mm * 128:(mm + 1) * 128],
                                        rhs=hT[:, kg * KT + k, sl],
                                        start=(kg == 0 and k == 0),
                                        stop=(kg == 3 and k == KT - 1))
                    for c in range(NCH):
                        sl = slice(c * CH, (c + 1) * CH)
                        for mm in range(2):
                            nc.vector.tensor_tensor(out=xT[:, 2 * qm + mm, sl],
                                                    in0=yps[2 * c + mm],
                                                    in1=xT[:, 2 * qm + mm, sl],
                                                    op=ALU.add)

            # ================= main =================
            for layer in range(DEPTH):
                xn = big.tile([128, KT, TOK], F8, tag="xn", name="xn")
                rmsnorm(xn, sx=SXA)
                qT = big.tile([128, KT, TOK], F8, tag="qbuf", name="qbuf")
                kT = big.tile([128, KT, TOK], F8, tag="kbuf", name="kbuf")
                vv = big.tile([128, BB, INNER], F8, tag="vbuf", name="vbuf")
                o_all = big.tile([128, KT, TOK], F8, tag="obuf", name="obuf")
                qk_project(qT, wq, layer, xn, TOK, cosq, sinq, crs=cq_a)
                qk_project(kT, wk, layer, xn, TOK, cosq, sinq, crs=ck_a)
                v_project(vv, wv, layer, xn, TOK, cross=False, vdesc=vdesc_a)
                wsb_o = out_project_staged(wo, layer)
                attention(qT, kT, vv, o_all, 128, cross=False,
                          tail=lambda c, p: out_project_chunk(wsb_o, o_all, odesc_a, c, p))

                xn = big.tile([128, KT, TOK], F8, tag="xn", name="xn")
                rmsnorm(xn, sx=SXA)
                qT = big.tile([128, KT, TOK], F8, tag="qbuf", name="qbuf")
                kTc = big.tile([128, KT, TOKC], F8, tag="kbuf", name="kbuf")
                vvc = big.tile([128, BB, INNER], F8, tag="vbuf", name="vbuf")
                o_all = big.tile([128, KT, TOK], F8, tag="obuf", name="obuf")
                qk_project(qT, wqc, layer, xn, TOK, cosq, sinq, crs=cq_c)
                qk_project(kTc, wkc, layer, csT, TOKC, cosc, sinc, crs=ck_c)
                v_project(vvc, wvc, layer, csT, TOKC, cross=True, vdesc=vdesc_c)
                wsb_oc = out_project_staged(woc, layer)
                attention(qT, kTc, vvc, o_all, M_CTX, cross=True,
                          tail=lambda c, p: out_project_chunk(wsb_oc, o_all, odesc_c, c, p))

                xn = big.tile([128, KT, TOK], BF, tag="xn", name="xn")
                rmsnorm(xn)
                ffn(layer, xn)

            rmsnorm(None, dst_f32=outT, gcol=None if gout_ones else gout_sb)

    nc.compile()
    return nc


_NC_CACHE = {}


def _get_nc(BB, DEPTH, scales, gelu_exact=True, has_bias=False):
    key = (BB, DEPTH, scales, gelu_exact, has_bias)
    if key not in _NC_CACHE:
        _NC_CACHE[key] = _build(BB, DEPTH, scales, gelu_exact, has_bias)
    return _NC_CACHE[key]


def _fp8_scale(w):
    """Largest power-of-2 scale keeping |w*s| <= 192 (e4m3 max finite 240)."""
    mx = float(np.abs(w).max())
    return float(2.0 ** np.floor(np.log2(192.0 / max(mx, 1e-30))))


def _stage2d(W, nblk):
    """[L, K, M] -> [L, nblk, 128, K//128, 512] staging layout (contiguous per
    partition per block, so each staging DMA is 128 linear descriptors)."""
    L, K, M = W.shape
    kt = K // 128
    assert M == nblk * 512
    return np.ascontiguousarray(
        W.reshape(L, kt, 128, nblk, 512).transpose(0, 3, 2, 1, 4))


def _stage_w2(W2):
    """[L, FF, DIM] -> [L, 4 kg, 4 qm, 128, KT, 256]."""
    L = W2.shape[0]
    return np.ascontiguousarray(
        W2.reshape(L, 4, KT, 128, 4, 256).transpose(0, 1, 4, 3, 2, 5))


def _to_fp8(w, s):
    q = (np.asarray(w, np.float32) * s).astype(E4)
    assert np.isfinite(q.astype(np.float32)).all()
    return q


def _rope_tables(n_pos, n_cols):
    """Masked full-head tables [128, n_cols]: rope rows (d%64<32) carry cos/sin,
    pass rows carry cos=1, sin=0. Token columns are batch-periodic."""
    inv = 1.0 / (10000.0 ** (np.arange(0, ROT, 2, dtype=np.float32) / ROT))  # [16]
    pos = np.arange(n_cols, dtype=np.float32) % n_pos
    d = np.arange(64)
    f = inv[d % 16]
    ang = f[:, None] * pos[None, :]
    cos = np.cos(ang)
    sin = np.sin(ang) * np.where(d % 32 < 16, -1.0, 1.0).astype(np.float32)[:, None]
    mask_rope = (d < 32)[:, None]
    cos = np.where(mask_rope, cos, 1.0)
    sin = np.where(mask_rope, sin, 0.0)
    return (np.tile(cos, (2, 1)).astype(BF16), np.tile(sin, (2, 1)).astype(BF16))


def _pack_qk(W):
    return W  # natural layout; rotation happens on-device via the shift matmul


def _shift_matrix():
    """S [128,128] bf16: out[m] = in[src(m)] for rope rows, 0 for pass rows."""
    S = np.zeros((128, 128), np.float32)
    for m in range(128):
        d = m % 64
        if d < 32:
            S[64 * (m // 64) + (d + 16) % 32, m] = 1.0
    return S.astype(BF16)


def _prep_weights(inputs, DEPTH):
    f32 = np.float32
    g_attn = np.asarray(inputs["g_attn"], f32)
    g_cross = np.asarray(inputs["g_cross"], f32)
    g_ff = np.asarray(inputs["g_ff"], f32)
    out = {}
    wq_l, wk_l, wv_l, wqc_l, wkc_l, wvc_l = [], [], [], [], [], []
    for L in range(DEPTH):
        Wq = np.asarray(inputs["Wq_a"][L], f32) * g_attn[L][:, None] * (DH ** -0.5)
        Wkv = np.asarray(inputs["Wkv_a"][L], f32) * g_attn[L][:, None]
        wq_l.append(_pack_qk(Wq))
        wk_l.append(_pack_qk(Wkv[:, :INNER]))
        wv_l.append(Wkv[:, INNER:])
        Wqc = np.asarray(inputs["Wq_c"][L], f32) * g_cross[L][:, None] * (DH ** -0.5)
        Wkvc = np.asarray(inputs["Wkv_c"][L], f32)  # context is not normed
        wqc_l.append(_pack_qk(Wqc))
        wkc_l.append(_pack_qk(Wkvc[:, :INNER]))
        wvc_l.append(Wkvc[:, INNER:])
    wo_f = np.asarray(inputs["Wo_a"], f32)[:DEPTH]
    woc_f = np.asarray(inputs["Wo_c"], f32)[:DEPTH]
    stacks = dict(wq=np.stack(wq_l), wk=np.stack(wk_l), wv=np.stack(wv_l),
                  wo=wo_f, wqc=np.stack(wqc_l), wkc=np.stack(wkc_l),
                  wvc=np.stack(wvc_l), woc=woc_f)
    scales = tuple(_fp8_scale(stacks[n]) for n in
                   ("wq", "wk", "wv", "wo", "wqc", "wkc", "wvc", "woc"))
    scales = scales + (bool(np.all(np.asarray(inputs["g_out"], f32) == 1.0)),)
    for n, s in zip(("wq", "wk", "wv", "wo", "wqc", "wkc", "wvc", "woc"), scales[:8]):
        out[n] = _stage2d(_to_fp8(stacks[n], s), 2)
    out["w1"] = _stage2d(
        (np.asarray(inputs["W1"], f32)[:DEPTH] * g_ff[:DEPTH, :, None]).astype(BF16), 8)
    out["w2"] = _stage_w2(np.asarray(inputs["W2"], f32)[:DEPTH].astype(BF16))
    out["b1c"] = np.ascontiguousarray(
        np.asarray(inputs["b1"], f32)[:DEPTH].reshape(DEPTH, FF // 128, 128).transpose(0, 2, 1))
    out["goutc"] = np.ascontiguousarray(np.asarray(inputs["g_out"], f32).reshape(KT, 128).T)
    brows = np.stack([np.asarray(inputs["bo_a"], f32)[:DEPTH],
                      np.asarray(inputs["bo_c"], f32)[:DEPTH],
                      np.asarray(inputs["b2"], f32)[:DEPTH]], axis=1)
    has_bias = bool(np.any(brows))
    assert not has_bias, "fp8 out_project path dropped the bias matmul"
    return out, has_bias, scales


def prepare(inputs, BB, DEPTH, n_cores):
    """Returns (in_maps, has_bias, scales) for n_cores cores."""
    TOK, TOKC = BB * N_CTX, BB * M_CTX
    CH, CHC = min(512, TOK), min(512, TOKC)
    w, has_bias, scales = _prep_weights(inputs, DEPTH)
    cosq, sinq = _rope_tables(N_CTX, CH)
    cosc, sinc = _rope_tables(M_CTX, CHC)
    x = np.asarray(inputs["x"], np.float32)
    cs = np.asarray(inputs["chunked_seq"], np.float32)
    in_maps = []
    for c in range(n_cores):
        xs = x[c * BB:(c + 1) * BB]
        css = cs[c * BB:(c + 1) * BB]
        m = dict(w)
        m["xT"] = np.ascontiguousarray(xs.reshape(TOK, DIM).T)
        m["csT"] = _to_fp8(np.ascontiguousarray(css.reshape(TOKC, DIM).T), SXC)
        m["cosq"], m["sinq"] = cosq, sinq
        m["shiftm"] = _shift_matrix()
        m["cosc"], m["sinc"] = cosc, sinc
        in_maps.append(m)
    return in_maps, has_bias, scales


def run_cores(inputs, BB, DEPTH, n_cores, gelu_exact=True):
    in_maps, has_bias, scales = prepare(inputs, BB, DEPTH, n_cores)
    nc = _get_nc(BB, DEPTH, scales, gelu_exact, has_bias)
    res = run_bass_kernel_spmd(nc, in_maps, list(range(n_cores)))
    outs = []
    for c in range(n_cores):
        oT = res.results[c]["outT"]
        outs.append(np.asarray(oT, np.float32).T.reshape(BB, N_CTX, DIM))
    return np.concatenate(outs, axis=0)


def kernel(**inputs):
    return run_cores(inputs, BB=8, DEPTH=3, n_cores=N_CORES).astype(np.float32)



# revision 90
# speedup vs baseline: 1.0005x; 1.0005x over previous
"""Trainium2 Bass kernel for a 3-layer transformer encoder (self+cross attention, FFN).

Data-parallel over batch: 64 batches split as 8 per NeuronCore.
Residual stream kept feature-major fp32 [DIM partitions, tokens].

Precision/perf strategy:
- All attention projections (Q/K/V/O, self+cross) run as fp8-e4m3 DoubleRow
  matmuls (two k-tiles contracted per instruction at 0.5 cycles/row), with
  power-of-2 per-tensor weight scales computed on the host and descale factors
  folded into existing activation copies / the softmax exp / the residual add.
- The FFN stays bf16 (fp8 there costs ~2e-2 rel err); its weights are staged
  once per layer in a contiguous per-partition dram layout, with hT holding
  both 512-token chunks so W2 streams only once.
- Attention computes sim transposed ([k, q] per head) so the fp8 exp output
  feeds attn@v directly (no transposes / psum round trips); softmax
  denominators come from an all-ones-stationary PE matmul broadcast across
  partitions, and the normalization is a reciprocal-multiply on the [dh, q]
  attention output. The out-projection is interleaved into the attention batch
  loop in 2-m-tile pieces to fill PE bubbles.
- RoPE: rotate-half is one PE matmul against a constant shift matrix; the two
  elementwise multiplies are split across Pool (cos, all-SBUF bf16) and DVE
  (sin, psum operand); rmsnorm statistics use fp8-DR matmuls and a
  recip+Sqrt tail (avoids the Ln/Exp act-table reload thrash).
"""

import numpy as np
import ml_dtypes

import concourse.mybir as mybir
import concourse.tile as tile
from concourse import bacc
from concourse.bass_utils import run_bass_kernel_spmd
from concourse.masks import make_identity

BF16 = ml_dtypes.bfloat16
E4 = ml_dtypes.float8_e4m3
F32 = mybir.dt.float32
BF = mybir.dt.bfloat16
F8 = mybir.dt.float8e4
DRMODE = mybir.MatmulPerfMode.DoubleRow
AF = mybir.ActivationFunctionType
ALU = mybir.AluOpType
AX = mybir.AxisListType

SXA = 32.0   # fp8 scale for rmsnormed activations
SXC = 32.0   # fp8 scale for raw chunked_seq
SO = 16.0    # fp8 scale for attention output (pre out-projection)
S8Q = 256.0  # fp8 scale for rope'd q
S8K = 16.0   # fp8 scale for rope'd k

DIM = 1024
HEADS = 16
DH = 64
INNER = 1024
FF = 4096
ROT = 32
N_CTX = 128   # query tokens per batch
M_CTX = 64    # chunk tokens per batch
N_CORES = 8
KT = DIM // 128  # 8 k-subtiles for DIM contraction


def _build(BB, DEPTH, scales, gelu_exact=True, has_bias=False):
    """Build + compile the per-core bass program for BB local batches, DEPTH layers."""
    TOK = BB * N_CTX
    TOKC = BB * M_CTX
    CH = min(512, TOK)
    NCH = TOK // CH
    CHC = min(512, TOKC)

    swq, swk, swv, swo, swqc, swkc, swvc, swoc, gout_ones = scales
    # rope outputs are rescaled into fp8 range via the STT scalar:
    cq_a = S8Q / (SXA * swq)
    ck_a = S8K / (SXA * swk)
    cq_c = S8Q / (SXA * swqc)
    ck_c = S8K / (SXC * swkc)
    exp_s = 1.0 / (S8Q * S8K)    # same for self and cross
    vdesc_a = SO / (SXA * swv)   # vv carries x16 so o_all = e@vv/d lands at xSO
    vdesc_c = SO / (SXC * swvc)
    odesc_a = 1.0 / (SO * swo)
    odesc_c = 1.0 / (SO * swoc)

    nc = bacc.Bacc("TRN2", target_bir_lowering=False, debug=False)

    xT_in = nc.dram_tensor("xT", [DIM, TOK], F32, kind="ExternalInput")
    csT_in = nc.dram_tensor("csT", [DIM, TOKC], F8, kind="ExternalInput")
    # staged-weight dram layout: [..., block, 128 partitions, KT, 512] so each
    # staging DMA reads one contiguous row per partition (128 descriptors).
    wq = nc.dram_tensor("wq", [DEPTH, 2, 128, KT, 512], F8, kind="ExternalInput")
    wk = nc.dram_tensor("wk", [DEPTH, 2, 128, KT, 512], F8, kind="ExternalInput")
    wv = nc.dram_tensor("wv", [DEPTH, 2, 128, KT, 512], F8, kind="ExternalInput")
    wo = nc.dram_tensor("wo", [DEPTH, 2, 128, KT, 512], F8, kind="ExternalInput")
    wqc = nc.dram_tensor("wqc", [DEPTH, 2, 128, KT, 512], F8, kind="ExternalInput")
    wkc = nc.dram_tensor("wkc", [DEPTH, 2, 128, KT, 512], F8, kind="ExternalInput")
    wvc = nc.dram_tensor("wvc", [DEPTH, 2, 128, KT, 512], F8, kind="ExternalInput")
    woc = nc.dram_tensor("woc", [DEPTH, 2, 128, KT, 512], F8, kind="ExternalInput")
    w1 = nc.dram_tensor("w1", [DEPTH, 8, 128, KT, 512], BF, kind="ExternalInput")
    w2 = nc.dram_tensor("w2", [DEPTH, 4, 4, 128, KT, 256], BF, kind="ExternalInput")
    b1c = nc.dram_tensor("b1c", [DEPTH, 128, FF // 128], F32, kind="ExternalInput")
    goutc = nc.dram_tensor("goutc", [128, KT], F32, kind="ExternalInput")
    shiftm_in = nc.dram_tensor("shiftm", [128, 128], BF, kind="ExternalInput")
    if has_bias:
        brows = nc.dram_tensor("brows", [DEPTH, 3, DIM], BF, kind="ExternalInput")
    cosq_in = nc.dram_tensor("cosq", [128, CH], BF, kind="ExternalInput")
    sinq_in = nc.dram_tensor("sinq", [128, CH], BF, kind="ExternalInput")
    cosc_in = nc.dram_tensor("cosc", [128, CHC], BF, kind="ExternalInput")
    sinc_in = nc.dram_tensor("sinc", [128, CHC], BF, kind="ExternalInput")

    outT = nc.dram_tensor("outT", [DIM, TOK], F32, kind="ExternalOutput")

    with tile.TileContext(nc) as tc:
        with tc.tile_pool(name="singles", bufs=1) as singles, \
             tc.tile_pool(name="big", bufs=1) as big, \
             tc.tile_pool(name="wst", bufs=3) as wst, \
             tc.tile_pool(name="scratch", bufs=2) as scratch, \
             tc.tile_pool(name="qrawp", bufs=1) as qrawp, \
             tc.tile_pool(name="attn", bufs=2) as attn_pool, \
             tc.tile_pool(name="psmm", bufs=5, space="PSUM") as psmm, \
             tc.tile_pool(name="psat", bufs=1, space="PSUM") as psat, \
             tc.tile_pool(name="psnrm", bufs=1, space="PSUM") as psnrm:

            # ---------- constants ----------
            ones_col = singles.tile([128, 1], BF)
            nc.vector.memset(ones_col, 1.0)
            ones8w = singles.tile([128, 2, 64], F8)
            nc.vector.memset(ones8w, 1.0)
            ones_mat = singles.tile([128, 128], BF)
            nc.vector.memset(ones_mat, 1.0)
            ones_row_b = singles.tile([1, CH], BF)
            nc.vector.memset(ones_row_b, 1.0)
            cosq = singles.tile([128, CH], BF)
            nc.sync.dma_start(out=cosq, in_=cosq_in[:])
            sinq = singles.tile([128, CH], BF)
            nc.sync.dma_start(out=sinq, in_=sinq_in[:])
            cosc = singles.tile([128, CHC], BF)
            nc.sync.dma_start(out=cosc, in_=cosc_in[:])
            sinc = singles.tile([128, CHC], BF)
            nc.sync.dma_start(out=sinc, in_=sinc_in[:])
            b1_sb = singles.tile([128, DEPTH, FF // 128], F32)
            nc.sync.dma_start(out=b1_sb, in_=b1c.rearrange("l p m -> p l m"))
            gout_sb = singles.tile([128, KT], F32)
            nc.sync.dma_start(out=gout_sb, in_=goutc[:])
            shiftm = singles.tile([128, 128], BF)
            nc.sync.dma_start(out=shiftm, in_=shiftm_in[:])
            if has_bias:
                brows_sb = singles.tile([1, DEPTH, 3, DIM], BF)
                nc.sync.dma_start(out=brows_sb, in_=brows[None])

            xT = singles.tile([128, KT, TOK], F32)
            in_engines = [nc.sync, nc.scalar, nc.gpsimd]
            for k in range(KT):
                in_engines[k % 3].dma_start(
                    out=xT[:, k],
                    in_=xT_in.rearrange("(kt p) t -> p kt t", p=128)[:, k])
            csT = singles.tile([128, KT, TOKC], F8)
            nc.scalar.dma_start(out=csT, in_=csT_in.rearrange("(kt p) t -> p kt t", p=128))

            # ---------- rmsnorm ----------
            def rmsnorm(dst, dst_f32=None, gcol=None, sx=1.0):
                sq = big.tile([128, KT, TOK], F8, tag="obuf", name="sq")  # obuf slot is dead here
                for c in range(NCH):
                    sl = slice(c * CH, (c + 1) * CH)
                    for k in range(KT):
                        if k % 3 == 0:
                            nc.scalar.activation(out=sq[:, k, sl], in_=xT[:, k, sl],
                                                 func=AF.Square)
                        elif k % 3 == 1:
                            nc.vector.tensor_tensor(out=sq[:, k, sl], in0=xT[:, k, sl],
                                                    in1=xT[:, k, sl], op=ALU.mult)
                        else:
                            nc.gpsimd.tensor_tensor(out=sq[:, k, sl], in0=xT[:, k, sl],
                                                    in1=xT[:, k, sl], op=ALU.mult)
                    sp = psnrm.tile([64, CH], F32, tag="nrm", name="nrm")
                    for k in range(0, KT, 2):
                        nc.tensor.matmul(sp, lhsT=ones8w, rhs=sq[:, k:k + 2, sl],
                                         start=(k == 0), stop=(k == KT - 2),
                                         perf_mode=DRMODE)
                    # rstd = sx/sqrt(ms) via DVE recip + ACT Sqrt (Sqrt/Copy/
                    # Square share act tables with everything, unlike Ln/Exp
                    # whose alternation forced a table reload per chunk)
                    rr = scratch.tile([1, CH], BF, tag="rstd", name="rr")
                    with nc.allow_low_precision(reason="bf16 rmsnorm recip"):
                        nc.vector.reciprocal(out=rr, in_=sp[0:1])
                    rstd_b = scratch.tile([1, CH], BF, tag="rstdb", name="rstd_b")
                    nc.scalar.activation(out=rstd_b, in_=rr, func=AF.Sqrt,
                                         scale=DIM * sx * sx, bias=0.0)
                    bp = psnrm.tile([128, CH], F32, tag="nrm", name="nrm")
                    nc.tensor.matmul(bp, lhsT=ones_row_b[:, :128], rhs=rstd_b,
                                     start=True, stop=True)
                    bp_sb = scratch.tile([128, CH], BF, tag="bpsb", name="bp_sb")
                    nc.scalar.copy(out=bp_sb, in_=bp)
                    for k in range(KT):
                        if dst is not None:
                            eng = nc.vector if k % 2 == 0 else nc.gpsimd
                            eng.tensor_tensor(out=dst[:, k, sl], in0=xT[:, k, sl],
                                              in1=bp_sb, op=ALU.mult)
                        if dst_f32 is not None:
                            of = scratch.tile([128, CH], F32, tag="outf", name="outf")
                            if gcol is None:  # g_out == 1: plain normalize
                                eng2 = nc.vector if k % 2 == 0 else nc.gpsimd
                                eng2.tensor_tensor(out=of, in0=xT[:, k, sl],
                                                   in1=bp_sb, op=ALU.mult)
                            else:
                                nc.vector.scalar_tensor_tensor(
                                    out=of, in0=xT[:, k, sl], scalar=gcol[:, k, None],
                                    in1=bp_sb, op0=ALU.mult, op1=ALU.mult)
                            eng = nc.sync if k % 2 == 0 else nc.scalar
                            eng.dma_start(out=dst_f32[k * 128:(k + 1) * 128, sl],
                                          in_=of)

            # ---------- rope-packed q/k projection (fp8 DoubleRow) ----------
            def qk_project(dst, wdram, layer, src, n_src_tok, cos_t, sin_t, crs):
                ch = min(512, n_src_tok)
                nch = n_src_tok // ch
                for sg in range(2):  # m-tiles 4*sg .. 4*sg+3
                    wn = wst.tile([128, KT, 512], F8, tag="wstage", name="wstage")
                    nc.sync.dma_start(out=wn, in_=wdram[layer, sg])
                    for c in range(nch):
                        sl = slice(c * ch, (c + 1) * ch)
                        qps, qraws = [], []
                        for m in range(4):
                            qp = psmm.tile([128, 512], F32, tag="mm", name="mm")[:, :ch]
                            for k in range(0, KT, 2):
                                nc.tensor.matmul(qp, lhsT=wn[:, k:k + 2, m * 128:(m + 1) * 128],
                                                 rhs=src[:, k:k + 2, sl],
                                                 start=(k == 0), stop=(k == KT - 2),
                                                 perf_mode=DRMODE)
                            # the fp8-range rescale rides the psum->sbuf copy
                            qraw = qrawp.tile([128, 512], BF, tag=f"qraw{m}",
                                                name=f"qraw{m}")[:, :ch]
                            nc.scalar.activation(out=qraw, in_=qp, func=AF.Copy,
                                                 scale=crs)
                            qps.append(qp); qraws.append(qraw)
                        for m in range(4):
                            rp = psat.tile([128, 512], F32, tag="op", name="op")[:, :ch]
                            nc.tensor.matmul(rp, lhsT=shiftm, rhs=qraws[m],
                                             start=True, stop=True)
                            # qraw·cos on Pool (all-SBUF), rp·sin on DVE
                            tcos = scratch.tile([128, 512], BF, tag="tcos", name="tcos")[:, :ch]
                            nc.gpsimd.tensor_tensor(out=tcos, in0=qraws[m],
                                                    in1=cos_t[:, :ch], op=ALU.mult)
                            tsin = scratch.tile([128, 512], BF, tag="tsin", name="tsin")[:, :ch]
                            nc.vector.tensor_tensor(out=tsin, in0=rp,
                                                    in1=sin_t[:, :ch], op=ALU.mult)
                            nc.gpsimd.tensor_tensor(out=dst[:, 4 * sg + m, sl],
                                                    in0=tcos, in1=tsin, op=ALU.add)

            # ---------- v projection (token-major, fp8 DoubleRow) ----------
            def v_project(dst, wdram, layer, src, n_src_tok, cross, vdesc):
                wsb = []
                for g in range(2):
                    wt = wst.tile([128, KT, 512], F8, tag="wstage", name="wstage")
                    nc.sync.dma_start(out=wt, in_=wdram[layer, g])
                    wsb.append(wt)
                for mt in range(n_src_tok // 128):
                    for g in range(2):
                        vp = psmm.tile([128, 512], F32, tag="mm", name="mm")
                        for k in range(0, KT, 2):
                            nc.tensor.matmul(vp, lhsT=src[:, k:k + 2, mt * 128:(mt + 1) * 128],
                                             rhs=wsb[g][:, k:k + 2, :],
                                             start=(k == 0), stop=(k == KT - 2),
                                             perf_mode=DRMODE)
                        if not cross:
                            nc.scalar.activation(out=dst[:, mt, g * 512:(g + 1) * 512],
                                                 in_=vp, func=AF.Copy, scale=vdesc)
                        else:
                            nc.scalar.activation(out=dst[0:64, 2 * mt, g * 512:(g + 1) * 512],
                                                 in_=vp[0:64], func=AF.Copy, scale=vdesc)
                            nc.scalar.activation(out=dst[64:128, 2 * mt + 1, g * 512:(g + 1) * 512],
                                                 in_=vp[64:128], func=AF.Copy, scale=vdesc)

            # ---------- attention core ([k,q]-layout, fp8 e, PE denominators) ----------
            def attention(qT, kT, vv, o_all, n_k, cross, tail=None):
                # sim is computed transposed (out [k, q] per head) so the exp
                # output feeds attn@v directly; softmax denominators come from
                # a ones-matmul over the k partitions; the divide happens on
                # the [dh, q] attention output (16x fewer elements than e).
                order = [2 * i for i in range(HEADS // 2)] + [2 * i + 1 for i in range(HEADS // 2)]
                pos = {h: i for i, h in enumerate(order)}

                def phase_a(b):
                    base = 64 * (b % 2) if cross else 0
                    qsl = slice(b * 128, (b + 1) * 128)
                    ksl = slice(b * n_k, (b + 1) * n_k)
                    e_b = attn_pool.tile([128, HEADS * 128], F8, tag="e_b", name="e_b")
                    for g in range(4):  # 4 heads per psum tile
                        heads = order[g * 4:(g + 1) * 4]
                        sp = psmm.tile([128, 512], F32, tag="mm", name="mm")
                        for j, h in enumerate(heads):
                            hb = 64 * (h % 2)
                            nc.tensor.matmul(
                                sp[base:base + n_k, j * 128:(j + 1) * 128],
                                lhsT=kT[hb:hb + 64, h // 2, ksl],
                                rhs=qT[hb:hb + 64, h // 2, qsl],
                                start=(j == 0), stop=(j == 3),
                                skip_group_check=True)
                        nc.scalar.activation(out=e_b[base:base + n_k, g * 512:(g + 1) * 512],
                                             in_=sp[base:base + n_k], func=AF.Exp,
                                             scale=exp_s)
                    return e_b

                def phase_b(b, e_b):
                    base = 64 * (b % 2) if cross else 0
                    qsl = slice(b * 128, (b + 1) * 128)
                    for half in range(2):
                        # denominators for this half's 8 heads, broadcast to all
                        # 64 partitions by an all-ones stationary: evens on top
                        # partitions, odds on bottom (matches op_ layout)
                        dD = psat.tile([128, 512], F32, tag="dD", name="dD")
                        for par in range(2):
                            esl = slice(par * 1024 + half * 512,
                                        par * 1024 + half * 512 + 512)
                            nc.tensor.matmul(dD[64 * par:64 * par + 64],
                                             lhsT=ones_mat[base:base + n_k, :64],
                                             rhs=e_b[base:base + n_k, esl],
                                             start=True, stop=True,
                                             skip_group_check=True)
                        dr = attn_pool.tile([128, 512], BF, tag="dr", name="dr")
                        with nc.allow_low_precision(reason="bf16 softmax denom"):
                            nc.vector.reciprocal(out=dr, in_=dD)
                        op_ = psat.tile([128, 512], F32, tag="op", name="op")
                        for hp in range(4):
                            h0 = 8 * half + 2 * hp
                            nc.tensor.matmul(
                                op_[0:64, hp * 128:(hp + 1) * 128],
                                lhsT=vv[base:base + n_k, b, h0 * 64:(h0 + 1) * 64],
                                rhs=e_b[base:base + n_k, pos[h0] * 128:(pos[h0] + 1) * 128],
                                start=(hp == 0), stop=False, tile_position=(base, 0),
                                skip_group_check=True)
                            nc.tensor.matmul(
                                op_[64:128, hp * 128:(hp + 1) * 128],
                                lhsT=vv[base:base + n_k, b, (h0 + 1) * 64:(h0 + 2) * 64],
                                rhs=e_b[base:base + n_k, pos[h0 + 1] * 128:(pos[h0 + 1] + 1) * 128],
                                start=(hp == 0), stop=(hp == 3), tile_position=(base, 64),
                                skip_group_check=True)
                        nc.vector.tensor_tensor(
                            out=o_all[:, 4 * half:4 * half + 4, qsl],
                            in0=op_.rearrange("p (hp t) -> p hp t", hp=4),
                            in1=dr.rearrange("p (hp t) -> p hp t", hp=4),
                            op=ALU.mult)

                prev = None
                for b in range(BB):
                    e_b = phase_a(b)
                    if prev is not None:
                        phase_b(prev[0], prev[1])
                        # once batches 0..3 (= token chunk 0) are done, spread
                        # the chunk-0 out-projection pieces over the remaining
                        # batch iterations to fill PE bubbles
                        if prev[0] >= 3 and tail is not None:
                            tail(0, prev[0] - 3)
                    prev = (b, e_b)
                phase_b(prev[0], prev[1])
                if tail is not None:
                    for piece in range(4):
                        tail(1, piece)

            # ---------- output projection + residual (fp8 DoubleRow) ----------
            def out_project_staged(wdram, layer):
                wsb = []
                for g in range(2):
                    wt = wst.tile([128, KT, 512], F8, tag="wstage", name="wstage")
                    nc.sync.dma_start(out=wt, in_=wdram[layer, g])
                    wsb.append(wt)
                return wsb

            def out_project_chunk(wsb, src, odesc, c, piece):
                sl = slice(c * CH, (c + 1) * CH)
                for m in range(2 * piece, 2 * piece + 2):
                    pp = psmm.tile([128, 512], F32, tag="mm", name="mm")[:, :CH]
                    for k in range(0, KT, 2):
                        nc.tensor.matmul(pp,
                                         lhsT=wsb[m // 4][:, k:k + 2, (m % 4) * 128:(m % 4 + 1) * 128],
                                         rhs=src[:, k:k + 2, sl],
                                         start=(k == 0), stop=(k == KT - 2),
                                         perf_mode=DRMODE)
                    nc.vector.scalar_tensor_tensor(out=xT[:, m, sl], in0=pp,
                                                   scalar=odesc, in1=xT[:, m, sl],
                                                   op0=ALU.mult, op1=ALU.add)

            # ---------- FFN (weights staged once; hT holds both chunks) ----------
            def ffn(layer, xn):
                gelu_f = AF.Gelu if gelu_exact else AF.Square
                hT = big.tile([128, FF // 128, TOK], BF, tag="obuf", name="obuf")
                for g in range(FF // 512):
                    wt = wst.tile([128, KT, 512], BF, tag="wstage", name="wstage")
                    nc.sync.dma_start(out=wt, in_=w1[layer, g])
                    for c in range(NCH):
                        sl = slice(c * CH, (c + 1) * CH)
                        for mm in range(4):
                            fm = 4 * g + mm
                            hp = psmm.tile([128, 512], F32, tag="mm", name="mm")[:, :CH]
                            for k in range(KT):
                                nc.tensor.matmul(hp, lhsT=wt[:, k, mm * 128:(mm + 1) * 128],
                                                 rhs=xn[:, k, sl],
                                                 start=(k == 0), stop=(k == KT - 1))
                            nc.scalar.activation(out=hT[:, fm, sl], in_=hp, func=gelu_f,
                                                 bias=b1_sb[:, layer, fm, None], scale=1.0)
                for qm in range(4):  # 256-wide output blocks, both chunks at once
                    yps = [psmm.tile([128, 512], F32, tag="mm", name="mm")[:, :CH]
                           for _ in range(4)]  # index 2*c + mm
                    for kg in range(4):
                        wt = wst.tile([128, KT, 256], BF, tag="wstage2", name="wstage2")
                        nc.sync.dma_start(out=wt, in_=w2[layer, kg, qm])
                        for c in range(NCH):
                            sl = slice(c * CH, (c + 1) * CH)
                            for k in range(KT):
                                for mm in range(2):
                                    nc.tensor.matmul(
                                        yps[2 * c + mm],
                                        lhsT=wt[:, k, mm * 128:(mm + 1) * 128],
                                        rhs=hT[:, kg * KT + k, sl],
                                        start=(kg == 0 and k == 0),
                                        stop=(kg == 3 and k == KT - 1))
                    for c in range(NCH):
                        sl = slice(c * CH, (c + 1) * CH)
                        for mm in range(2):
                            nc.vector.tensor_tensor(out=xT[:, 2 * qm + mm, sl],
                                                    in0=yps[2 * c + mm],
                                                    in1=xT[:, 2 * qm + mm, sl],
                                                    op=ALU.add)

            # ================= main =================
            for layer in range(DEPTH):
                xn = big.tile([128, KT, TOK], F8, tag="xn", name="xn")
                rmsnorm(xn, sx=SXA)
                qT = big.tile([128, KT, TOK], F8, tag="qbuf", name="qbuf")
                kT = big.tile([128, KT, TOK], F8, tag="kbuf", name="kbuf")
                vv = big.tile([128, BB, INNER], F8, tag="vbuf", name="vbuf")
                o_all = big.tile([128, KT, TOK], F8, tag="obuf", name="obuf")
                qk_project(qT, wq, layer, xn, TOK, cosq, sinq, crs=cq_a)
                qk_project(kT, wk, layer, xn, TOK, cosq, sinq, crs=ck_a)
                v_project(vv, wv, layer, xn, TOK, cross=False, vdesc=vdesc_a)
                wsb_o = out_project_staged(wo, layer)
                attention(qT, kT, vv, o_all, 128, cross=False,
                          tail=lambda c, p: out_project_chunk(wsb_o, o_all, odesc_a, c, p))

                xn = big.tile([128, KT, TOK], F8, tag="xn", name="xn")
                rmsnorm(xn, sx=SXA)
                qT = big.tile([128, KT, TOK], F8, tag="qbuf", name="qbuf")
                kTc = big.tile([128, KT, TOKC], F8, tag="kbuf", name="kbuf")
                vvc = big.tile([128, BB, INNER], F8, tag="vbuf", name="vbuf")
                o_all = big.tile([128, KT, TOK], F8, tag="obuf", name="obuf")
                qk_project(qT, wqc, layer, xn, TOK, cosq, sinq, crs=cq_c)
                qk_project(kTc, wkc, layer, csT, TOKC, cosc, sinc, crs=ck_c)
                v_project(vvc, wvc, layer, csT, TOKC, cross=True, vdesc=vdesc_c)
                wsb_oc = out_project_staged(woc, layer)
                attention(qT, kTc, vvc, o_all, M_CTX, cross=True,
                          tail=lambda c, p: out_project_chunk(wsb_oc, o_all, odesc_c, c, p))

                xn = big.tile([128, KT, TOK], BF, tag="xn", name="xn")
                rmsnorm(xn)
                ffn(layer, xn)

            rmsnorm(None, dst_f32=outT, gcol=None if gout_ones else gout_sb)

    nc.compile()
    return nc


_NC_CACHE = {}


def _get_nc(BB, DEPTH, scales, gelu_exact=True, has_bias=False):
    key = (BB, DEPTH, scales, gelu_exact, has_bias)
    if key not in _NC_CACHE:
        _NC_CACHE[key] = _build(BB, DEPTH, scales, gelu_exact, has_bias)
    return _NC_CACHE[key]


def _fp8_scale(w):
    """Largest power-of-2 scale keeping |w*s| <= 192 (e4m3 max finite 240)."""
    mx = float(np.abs(w).max())
    return float(2.0 ** np.floor(np.log2(192.0 / max(mx, 1e-30))))


def _stage2d(W, nblk):
    """[L, K, M] -> [L, nblk, 128, K//128, 512] staging layout (contiguous per
    partition per block, so each staging DMA is 128 linear descriptors)."""
    L, K, M = W.shape
    kt = K // 128
    assert M == nblk * 512
    return np.ascontiguousarray(
        W.reshape(L, kt, 128, nblk, 512).transpose(0, 3, 2, 1, 4))


def _stage_w2(W2):
    """[L, FF, DIM] -> [L, 4 kg, 4 qm, 128, KT, 256]."""
    L = W2.shape[0]
    return np.ascontiguousarray(
        W2.reshape(L, 4, KT, 128, 4, 256).transpose(0, 1, 4, 3, 2, 5))


def _to_fp8(w, s):
    q = (np.asarray(w, np.float32) * s).astype(E4)
    assert np.isfinite(q.astype(np.float32)).all()
    return q


def _rope_tables(n_pos, n_cols):
    """Masked full-head tables [128, n_cols]: rope rows (d%64<32) carry cos/sin,
    pass rows carry cos=1, sin=0. Token columns are batch-periodic."""
    inv = 1.0 / (10000.0 ** (np.arange(0, ROT, 2, dtype=np.float32) / ROT))  # [16]
    pos = np.arange(n_cols, dtype=np.float32) % n_pos
    d = np.arange(64)
    f = inv[d % 16]
    ang = f[:, None] * pos[None, :]
    cos = np.cos(ang)
    sin = np.sin(ang) * np.where(d % 32 < 16, -1.0, 1.0).astype(np.float32)[:, None]
    mask_rope = (d < 32)[:, None]
    cos = np.where(mask_rope, cos, 1.0)
    sin = np.where(mask_rope, sin, 0.0)
    return (np.tile(cos, (2, 1)).astype(BF16), np.tile(sin, (2, 1)).astype(BF16))


def _pack_qk(W):
    return W  # natural layout; rotation happens on-device via the shift matmul


def _shift_matrix():
    """S [128,128] bf16: out[m] = in[src(m)] for rope rows, 0 for pass rows."""
    S = np.zeros((128, 128), np.float32)
    for m in range(128):
        d = m % 64
        if d < 32:
            S[64 * (m // 64) + (d + 16) % 32, m] = 1.0
    return S.astype(BF16)


def _prep_weights(inputs, DEPTH):
    f32 = np.float32
    g_attn = np.asarray(inputs["g_attn"], f32)
    g_cross = np.asarray(inputs["g_cross"], f32)
    g_ff = np.asarray(inputs["g_ff"], f32)
    out = {}
    wq_l, wk_l, wv_l, wqc_l, wkc_l, wvc_l = [], [], [], [], [], []
    for L in range(DEPTH):
        Wq = np.asarray(inputs["Wq_a"][L], f32) * g_attn[L][:, None] * (DH ** -0.5)
        Wkv = np.asarray(inputs["Wkv_a"][L], f32) * g_attn[L][:, None]
        wq_l.append(_pack_qk(Wq))
        wk_l.append(_pack_qk(Wkv[:, :INNER]))
        wv_l.append(Wkv[:, INNER:])
        Wqc = np.asarray(inputs["Wq_c"][L], f32) * g_cross[L][:, None] * (DH ** -0.5)
        Wkvc = np.asarray(inputs["Wkv_c"][L], f32)  # context is not normed
        wqc_l.append(_pack_qk(Wqc))
        wkc_l.append(_pack_qk(Wkvc[:, :INNER]))
        wvc_l.append(Wkvc[:, INNER:])
    wo_f = np.asarray(inputs["Wo_a"], f32)[:DEPTH]
    woc_f = np.asarray(inputs["Wo_c"], f32)[:DEPTH]
    stacks = dict(wq=np.stack(wq_l), wk=np.stack(wk_l), wv=np.stack(wv_l),
                  wo=wo_f, wqc=np.stack(wqc_l), wkc=np.stack(wkc_l),
                  wvc=np.stack(wvc_l), woc=woc_f)
    scales = tuple(_fp8_scale(stacks[n]) for n in
                   ("wq", "wk", "wv", "wo", "wqc", "wkc", "wvc", "woc"))
    scales = scales + (bool(np.all(np.asarray(inputs["g_out"], f32) == 1.0)),)
    for n, s in zip(("wq", "wk", "wv", "wo", "wqc", "wkc", "wvc", "woc"), scales[:8]):
        out[n] = _stage2d(_to_fp8(stacks[n], s), 2)
    out["w1"] = _stage2d(
        (np.asarray(inputs["W1"], f32)[:DEPTH] * g_ff[:DEPTH, :, None]).astype(BF16), 8)
    out["w2"] = _stage_w2(np.asarray(inputs["W2"], f32)[:DEPTH].astype(BF16))
    out["b1c"] = np.ascontiguousarray(
        np.asarray(inputs["b1"], f32)[:DEPTH].reshape(DEPTH, FF // 128, 128).transpose(0, 2, 1))
    out["goutc"] = np.ascontiguousarray(np.asarray(inputs["g_out"], f32).reshape(KT, 128).T)
    brows = np.stack([np.asarray(inputs["bo_a"], f32)[:DEPTH],
                      np.asarray(inputs["bo_c"], f32)[:DEPTH],
                      np.asarray(inputs["b2"], f32)[:DEPTH]], axis=1)
    has_bias = bool(np.any(brows))
    assert not has_bias, "fp8 out_project path dropped the bias matmul"
    return out, has_bias, scales


def prepare(inputs, BB, DEPTH, n_cores):
    """Returns (in_maps, has_bias, scales) for n_cores cores."""
    TOK, TOKC = BB * N_CTX, BB * M_CTX
    CH, CHC = min(512, TOK), min(512, TOKC)
    w, has_bias, scales = _prep_weights(inputs, DEPTH)
    cosq, sinq = _rope_tables(N_CTX, CH)
    cosc, sinc = _rope_tables(M_CTX, CHC)
    x = np.asarray(inputs["x"], np.float32)
    cs = np.asarray(inputs["chunked_seq"], np.float32)
    in_maps = []
    for c in range(n_cores):
        xs = x[c * BB:(c + 1) * BB]
        css = cs[c * BB:(c + 1) * BB]
        m = dict(w)
        m["xT"] = np.ascontiguousarray(xs.reshape(TOK, DIM).T)
        m["csT"] = _to_fp8(np.ascontiguousarray(css.reshape(TOKC, DIM).T), SXC)
        m["cosq"], m["sinq"] = cosq, sinq
        m["shiftm"] = _shift_matrix()
        m["cosc"], m["sinc"] = cosc, sinc
        in_maps.append(m)
    return in_maps, has_bias, scales


def run_cores(inputs, BB, DEPTH, n_cores, gelu_exact=True):
    in_maps, has_bias, scales = prepare(inputs, BB, DEPTH, n_cores)
    nc = _get_nc(BB, DEPTH, scales, gelu_exact, has_bias)
    res = run_bass_kernel_spmd(nc, in_maps, list(range(n_cores)))
    outs = []
    for c in range(n_cores):
        oT = res.results[c]["outT"]
        outs.append(np.asarray(oT, np.float32).T.reshape(BB, N_CTX, DIM))
    return np.concatenate(outs, axis=0)


def kernel(**inputs):
    return run_cores(inputs, BB=8, DEPTH=3, n_cores=N_CORES).astype(np.float32)



# revision 91
# speedup vs baseline: 1.0015x; 1.0010x over previous
"""Trainium2 Bass kernel for a 3-layer transformer encoder (self+cross attention, FFN).

Data-parallel over batch: 64 batches split as 8 per NeuronCore.
Residual stream kept feature-major fp32 [DIM partitions, tokens].

Precision/perf strategy:
- All attention projections (Q/K/V/O, self+cross) run as fp8-e4m3 DoubleRow
  matmuls (two k-tiles contracted per instruction at 0.5 cycles/row), with
  power-of-2 per-tensor weight scales computed on the host and descale factors
  folded into existing activation copies / the softmax exp / the residual add.
- The FFN stays bf16 (fp8 there costs ~2e-2 rel err); its weights are staged
  once per layer in a contiguous per-partition dram layout, with hT holding
  both 512-token chunks so W2 streams only once.
- Attention computes sim transposed ([k, q] per head) so the fp8 exp output
  feeds attn@v directly (no transposes / psum round trips); softmax
  denominators come from an all-ones-stationary PE matmul broadcast across
  partitions, and the normalization is a reciprocal-multiply on the [dh, q]
  attention output. The out-projection is interleaved into the attention batch
  loop in 2-m-tile pieces to fill PE bubbles.
- RoPE: rotate-half is one PE matmul against a constant shift matrix; the two
  elementwise multiplies are split across Pool (cos, all-SBUF bf16) and DVE
  (sin, psum operand); rmsnorm statistics use fp8-DR matmuls and a
  recip+Sqrt tail (avoids the Ln/Exp act-table reload thrash).
"""

import numpy as np
import ml_dtypes

import concourse.mybir as mybir
import concourse.tile as tile
from concourse import bacc
from concourse.bass_utils import run_bass_kernel_spmd
from concourse.masks import make_identity

BF16 = ml_dtypes.bfloat16
E4 = ml_dtypes.float8_e4m3
F32 = mybir.dt.float32
BF = mybir.dt.bfloat16
F8 = mybir.dt.float8e4
DRMODE = mybir.MatmulPerfMode.DoubleRow
AF = mybir.ActivationFunctionType
ALU = mybir.AluOpType
AX = mybir.AxisListType

SXA = 32.0   # fp8 scale for rmsnormed activations
SXC = 32.0   # fp8 scale for raw chunked_seq
SO = 16.0    # fp8 scale for attention output (pre out-projection)
S8Q = 256.0  # fp8 scale for rope'd q
S8K = 16.0   # fp8 scale for rope'd k

DIM = 1024
HEADS = 16
DH = 64
INNER = 1024
FF = 4096
ROT = 32
N_CTX = 128   # query tokens per batch
M_CTX = 64    # chunk tokens per batch
N_CORES = 8
KT = DIM // 128  # 8 k-subtiles for DIM contraction


def _build(BB, DEPTH, scales, gelu_exact=True, has_bias=False):
    """Build + compile the per-core bass program for BB local batches, DEPTH layers."""
    TOK = BB * N_CTX
    TOKC = BB * M_CTX
    CH = min(512, TOK)
    NCH = TOK // CH
    CHC = min(512, TOKC)

    swq, swk, swv, swo, swqc, swkc, swvc, swoc, gout_ones = scales
    # rope outputs are rescaled into fp8 range via the STT scalar:
    cq_a = S8Q / (SXA * swq)
    ck_a = S8K / (SXA * swk)
    cq_c = S8Q / (SXA * swqc)
    ck_c = S8K / (SXC * swkc)
    exp_s = 1.0 / (S8Q * S8K)    # same for self and cross
    vdesc_a = SO / (SXA * swv)   # vv carries x16 so o_all = e@vv/d lands at xSO
    vdesc_c = SO / (SXC * swvc)
    odesc_a = 1.0 / (SO * swo)
    odesc_c = 1.0 / (SO * swoc)

    nc = bacc.Bacc("TRN2", target_bir_lowering=False, debug=False)

    xT_in = nc.dram_tensor("xT", [DIM, TOK], F32, kind="ExternalInput")
    csT_in = nc.dram_tensor("csT", [DIM, TOKC], F8, kind="ExternalInput")
    # staged-weight dram layout: [..., block, 128 partitions, KT, 512] so each
    # staging DMA reads one contiguous row per partition (128 descriptors).
    wq = nc.dram_tensor("wq", [DEPTH, 2, 128, KT, 512], F8, kind="ExternalInput")
    wk = nc.dram_tensor("wk", [DEPTH, 2, 128, KT, 512], F8, kind="ExternalInput")
    wv = nc.dram_tensor("wv", [DEPTH, 2, 128, KT, 512], F8, kind="ExternalInput")
    wo = nc.dram_tensor("wo", [DEPTH, 2, 128, KT, 512], F8, kind="ExternalInput")
    wqc = nc.dram_tensor("wqc", [DEPTH, 2, 128, KT, 512], F8, kind="ExternalInput")
    wkc = nc.dram_tensor("wkc", [DEPTH, 2, 128, KT, 512], F8, kind="ExternalInput")
    wvc = nc.dram_tensor("wvc", [DEPTH, 2, 128, KT, 512], F8, kind="ExternalInput")
    woc = nc.dram_tensor("woc", [DEPTH, 2, 128, KT, 512], F8, kind="ExternalInput")
    w1 = nc.dram_tensor("w1", [DEPTH, 8, 128, KT, 512], BF, kind="ExternalInput")
    w2 = nc.dram_tensor("w2", [DEPTH, 4, 4, 128, KT, 256], BF, kind="ExternalInput")
    b1c = nc.dram_tensor("b1c", [DEPTH, 128, FF // 128], F32, kind="ExternalInput")
    goutc = nc.dram_tensor("goutc", [128, KT], F32, kind="ExternalInput")
    shiftm_in = nc.dram_tensor("shiftm", [128, 128], BF, kind="ExternalInput")
    if has_bias:
        brows = nc.dram_tensor("brows", [DEPTH, 3, DIM], BF, kind="ExternalInput")
    cosq_in = nc.dram_tensor("cosq", [128, CH], BF, kind="ExternalInput")
    sinq_in = nc.dram_tensor("sinq", [128, CH], BF, kind="ExternalInput")
    cosc_in = nc.dram_tensor("cosc", [128, CHC], BF, kind="ExternalInput")
    sinc_in = nc.dram_tensor("sinc", [128, CHC], BF, kind="ExternalInput")

    outT = nc.dram_tensor("outT", [DIM, TOK], F32, kind="ExternalOutput")

    with tile.TileContext(nc) as tc:
        with tc.tile_pool(name="singles", bufs=1) as singles, \
             tc.tile_pool(name="big", bufs=1) as big, \
             tc.tile_pool(name="wst", bufs=3) as wst, \
             tc.tile_pool(name="scratch", bufs=2) as scratch, \
             tc.tile_pool(name="qrawp", bufs=1) as qrawp, \
             tc.tile_pool(name="attn", bufs=2) as attn_pool, \
             tc.tile_pool(name="psmm", bufs=5, space="PSUM") as psmm, \
             tc.tile_pool(name="psat", bufs=1, space="PSUM") as psat, \
             tc.tile_pool(name="psnrm", bufs=1, space="PSUM") as psnrm:

            # ---------- constants ----------
            ones_col = singles.tile([128, 1], BF)
            nc.vector.memset(ones_col, 1.0)
            ones8w = singles.tile([128, 2, 64], F8)
            nc.vector.memset(ones8w, 1.0)
            ones_mat = singles.tile([128, 128], BF)
            nc.vector.memset(ones_mat, 1.0)
            ones_row_b = singles.tile([1, CH], BF)
            nc.vector.memset(ones_row_b, 1.0)
            cosq = singles.tile([128, CH], BF)
            nc.sync.dma_start(out=cosq, in_=cosq_in[:])
            sinq = singles.tile([128, CH], BF)
            nc.sync.dma_start(out=sinq, in_=sinq_in[:])
            cosc = singles.tile([128, CHC], BF)
            nc.sync.dma_start(out=cosc, in_=cosc_in[:])
            sinc = singles.tile([128, CHC], BF)
            nc.sync.dma_start(out=sinc, in_=sinc_in[:])
            b1_sb = singles.tile([128, DEPTH, FF // 128], F32)
            nc.sync.dma_start(out=b1_sb, in_=b1c.rearrange("l p m -> p l m"))
            gout_sb = singles.tile([128, KT], F32)
            nc.sync.dma_start(out=gout_sb, in_=goutc[:])
            shiftm = singles.tile([128, 128], BF)
            nc.sync.dma_start(out=shiftm, in_=shiftm_in[:])
            if has_bias:
                brows_sb = singles.tile([1, DEPTH, 3, DIM], BF)
                nc.sync.dma_start(out=brows_sb, in_=brows[None])

            xT = singles.tile([128, KT, TOK], F32)
            in_engines = [nc.sync, nc.scalar, nc.gpsimd]
            for k in range(KT):
                in_engines[k % 3].dma_start(
                    out=xT[:, k],
                    in_=xT_in.rearrange("(kt p) t -> p kt t", p=128)[:, k])
            csT = singles.tile([128, KT, TOKC], F8)
            nc.scalar.dma_start(out=csT, in_=csT_in.rearrange("(kt p) t -> p kt t", p=128))

            # ---------- rmsnorm ----------
            def rmsnorm(dst, dst_f32=None, gcol=None, sx=1.0):
                sq = big.tile([128, KT, TOK], F8, tag="obuf", name="sq")  # obuf slot is dead here
                for c in range(NCH):
                    sl = slice(c * CH, (c + 1) * CH)
                    for k in range(KT):
                        if k % 3 == 2:  # ACT is the boundary bottleneck: 2 ops
                            nc.scalar.activation(out=sq[:, k, sl], in_=xT[:, k, sl],
                                                 func=AF.Square)
                        elif k % 3 == 1:
                            nc.vector.tensor_tensor(out=sq[:, k, sl], in0=xT[:, k, sl],
                                                    in1=xT[:, k, sl], op=ALU.mult)
                        else:
                            nc.gpsimd.tensor_tensor(out=sq[:, k, sl], in0=xT[:, k, sl],
                                                    in1=xT[:, k, sl], op=ALU.mult)
                    sp = psnrm.tile([64, CH], F32, tag="nrm", name="nrm")
                    for k in range(0, KT, 2):
                        nc.tensor.matmul(sp, lhsT=ones8w, rhs=sq[:, k:k + 2, sl],
                                         start=(k == 0), stop=(k == KT - 2),
                                         perf_mode=DRMODE)
                    # rstd = sx/sqrt(ms) via DVE recip + ACT Sqrt (Sqrt/Copy/
                    # Square share act tables with everything, unlike Ln/Exp
                    # whose alternation forced a table reload per chunk)
                    rr = scratch.tile([1, CH], BF, tag="rstd", name="rr")
                    with nc.allow_low_precision(reason="bf16 rmsnorm recip"):
                        nc.vector.reciprocal(out=rr, in_=sp[0:1])
                    rstd_b = scratch.tile([1, CH], BF, tag="rstdb", name="rstd_b")
                    nc.scalar.activation(out=rstd_b, in_=rr, func=AF.Sqrt,
                                         scale=DIM * sx * sx, bias=0.0)
                    bp = psnrm.tile([128, CH], F32, tag="nrm", name="nrm")
                    nc.tensor.matmul(bp, lhsT=ones_row_b[:, :128], rhs=rstd_b,
                                     start=True, stop=True)
                    bp_sb = scratch.tile([128, CH], BF, tag="bpsb", name="bp_sb")
                    nc.scalar.copy(out=bp_sb, in_=bp)
                    for k in range(KT):
                        if dst is not None:
                            eng = nc.vector if k % 2 == 0 else nc.gpsimd
                            eng.tensor_tensor(out=dst[:, k, sl], in0=xT[:, k, sl],
                                              in1=bp_sb, op=ALU.mult)
                        if dst_f32 is not None:
                            of = scratch.tile([128, CH], F32, tag="outf", name="outf")
                            if gcol is None:  # g_out == 1: plain normalize
                                eng2 = nc.vector if k % 2 == 0 else nc.gpsimd
                                eng2.tensor_tensor(out=of, in0=xT[:, k, sl],
                                                   in1=bp_sb, op=ALU.mult)
                            else:
                                nc.vector.scalar_tensor_tensor(
                                    out=of, in0=xT[:, k, sl], scalar=gcol[:, k, None],
                                    in1=bp_sb, op0=ALU.mult, op1=ALU.mult)
                            eng = nc.sync if k % 2 == 0 else nc.scalar
                            eng.dma_start(out=dst_f32[k * 128:(k + 1) * 128, sl],
                                          in_=of)

            # ---------- rope-packed q/k projection (fp8 DoubleRow) ----------
            def qk_project(dst, wdram, layer, src, n_src_tok, cos_t, sin_t, crs):
                ch = min(512, n_src_tok)
                nch = n_src_tok // ch
                for sg in range(2):  # m-tiles 4*sg .. 4*sg+3
                    wn = wst.tile([128, KT, 512], F8, tag="wstage", name="wstage")
                    nc.sync.dma_start(out=wn, in_=wdram[layer, sg])
                    for c in range(nch):
                        sl = slice(c * ch, (c + 1) * ch)
                        qps, qraws = [], []
                        for m in range(4):
                            qp = psmm.tile([128, 512], F32, tag="mm", name="mm")[:, :ch]
                            for k in range(0, KT, 2):
                                nc.tensor.matmul(qp, lhsT=wn[:, k:k + 2, m * 128:(m + 1) * 128],
                                                 rhs=src[:, k:k + 2, sl],
                                                 start=(k == 0), stop=(k == KT - 2),
                                                 perf_mode=DRMODE)
                            # the fp8-range rescale rides the psum->sbuf copy
                            qraw = qrawp.tile([128, 512], BF, tag=f"qraw{m}",
                                                name=f"qraw{m}")[:, :ch]
                            nc.scalar.activation(out=qraw, in_=qp, func=AF.Copy,
                                                 scale=crs)
                            qps.append(qp); qraws.append(qraw)
                        for m in range(4):
                            rp = psat.tile([128, 512], F32, tag="op", name="op")[:, :ch]
                            nc.tensor.matmul(rp, lhsT=shiftm, rhs=qraws[m],
                                             start=True, stop=True)
                            # qraw·cos on Pool (all-SBUF), rp·sin on DVE
                            tcos = scratch.tile([128, 512], BF, tag="tcos", name="tcos")[:, :ch]
                            nc.gpsimd.tensor_tensor(out=tcos, in0=qraws[m],
                                                    in1=cos_t[:, :ch], op=ALU.mult)
                            tsin = scratch.tile([128, 512], BF, tag="tsin", name="tsin")[:, :ch]
                            nc.vector.tensor_tensor(out=tsin, in0=rp,
                                                    in1=sin_t[:, :ch], op=ALU.mult)
                            nc.gpsimd.tensor_tensor(out=dst[:, 4 * sg + m, sl],
                                                    in0=tcos, in1=tsin, op=ALU.add)

            # ---------- v projection (token-major, fp8 DoubleRow) ----------
            def v_project(dst, wdram, layer, src, n_src_tok, cross, vdesc):
                wsb = []
                for g in range(2):
                    wt = wst.tile([128, KT, 512], F8, tag="wstage", name="wstage")
                    nc.sync.dma_start(out=wt, in_=wdram[layer, g])
                    wsb.append(wt)
                for mt in range(n_src_tok // 128):
                    for g in range(2):
                        vp = psmm.tile([128, 512], F32, tag="mm", name="mm")
                        for k in range(0, KT, 2):
                            nc.tensor.matmul(vp, lhsT=src[:, k:k + 2, mt * 128:(mt + 1) * 128],
                                             rhs=wsb[g][:, k:k + 2, :],
                                             start=(k == 0), stop=(k == KT - 2),
                                             perf_mode=DRMODE)
                        if not cross:
                            nc.scalar.activation(out=dst[:, mt, g * 512:(g + 1) * 512],
                                                 in_=vp, func=AF.Copy, scale=vdesc)
                        else:
                            nc.scalar.activation(out=dst[0:64, 2 * mt, g * 512:(g + 1) * 512],
                                                 in_=vp[0:64], func=AF.Copy, scale=vdesc)
                            nc.scalar.activation(out=dst[64:128, 2 * mt + 1, g * 512:(g + 1) * 512],
                                                 in_=vp[64:128], func=AF.Copy, scale=vdesc)

            # ---------- attention core ([k,q]-layout, fp8 e, PE denominators) ----------
            def attention(qT, kT, vv, o_all, n_k, cross, tail=None):
                # sim is computed transposed (out [k, q] per head) so the exp
                # output feeds attn@v directly; softmax denominators come from
                # a ones-matmul over the k partitions; the divide happens on
                # the [dh, q] attention output (16x fewer elements than e).
                order = [2 * i for i in range(HEADS // 2)] + [2 * i + 1 for i in range(HEADS // 2)]
                pos = {h: i for i, h in enumerate(order)}

                def phase_a(b):
                    base = 64 * (b % 2) if cross else 0
                    qsl = slice(b * 128, (b + 1) * 128)
                    ksl = slice(b * n_k, (b + 1) * n_k)
                    e_b = attn_pool.tile([128, HEADS * 128], F8, tag="e_b", name="e_b")
                    for g in range(4):  # 4 heads per psum tile
                        heads = order[g * 4:(g + 1) * 4]
                        sp = psmm.tile([128, 512], F32, tag="mm", name="mm")
                        for j, h in enumerate(heads):
                            hb = 64 * (h % 2)
                            nc.tensor.matmul(
                                sp[base:base + n_k, j * 128:(j + 1) * 128],
                                lhsT=kT[hb:hb + 64, h // 2, ksl],
                                rhs=qT[hb:hb + 64, h // 2, qsl],
                                start=(j == 0), stop=(j == 3),
                                skip_group_check=True)
                        nc.scalar.activation(out=e_b[base:base + n_k, g * 512:(g + 1) * 512],
                                             in_=sp[base:base + n_k], func=AF.Exp,
                                             scale=exp_s)
                    return e_b

                def phase_b(b, e_b):
                    base = 64 * (b % 2) if cross else 0
                    qsl = slice(b * 128, (b + 1) * 128)
                    for half in range(2):
                        # denominators for this half's 8 heads, broadcast to all
                        # 64 partitions by an all-ones stationary: evens on top
                        # partitions, odds on bottom (matches op_ layout)
                        dD = psat.tile([128, 512], F32, tag="dD", name="dD")
                        for par in range(2):
                            esl = slice(par * 1024 + half * 512,
                                        par * 1024 + half * 512 + 512)
                            nc.tensor.matmul(dD[64 * par:64 * par + 64],
                                             lhsT=ones_mat[base:base + n_k, :64],
                                             rhs=e_b[base:base + n_k, esl],
                                             start=True, stop=True,
                                             skip_group_check=True)
                        dr = attn_pool.tile([128, 512], BF, tag="dr", name="dr")
                        with nc.allow_low_precision(reason="bf16 softmax denom"):
                            nc.vector.reciprocal(out=dr, in_=dD)
                        op_ = psat.tile([128, 512], F32, tag="op", name="op")
                        for hp in range(4):
                            h0 = 8 * half + 2 * hp
                            nc.tensor.matmul(
                                op_[0:64, hp * 128:(hp + 1) * 128],
                                lhsT=vv[base:base + n_k, b, h0 * 64:(h0 + 1) * 64],
                                rhs=e_b[base:base + n_k, pos[h0] * 128:(pos[h0] + 1) * 128],
                                start=(hp == 0), stop=False, tile_position=(base, 0),
                                skip_group_check=True)
                            nc.tensor.matmul(
                                op_[64:128, hp * 128:(hp + 1) * 128],
                                lhsT=vv[base:base + n_k, b, (h0 + 1) * 64:(h0 + 2) * 64],
                                rhs=e_b[base:base + n_k, pos[h0 + 1] * 128:(pos[h0 + 1] + 1) * 128],
                                start=(hp == 0), stop=(hp == 3), tile_position=(base, 64),
                                skip_group_check=True)
                        nc.vector.tensor_tensor(
                            out=o_all[:, 4 * half:4 * half + 4, qsl],
                            in0=op_.rearrange("p (hp t) -> p hp t", hp=4),
                            in1=dr.rearrange("p (hp t) -> p hp t", hp=4),
                            op=ALU.mult)

                prev = None
                for b in range(BB):
                    e_b = phase_a(b)
                    if prev is not None:
                        phase_b(prev[0], prev[1])
                        # once batches 0..3 (= token chunk 0) are done, spread
                        # the chunk-0 out-projection pieces over the remaining
                        # batch iterations to fill PE bubbles
                        if prev[0] >= 3 and tail is not None:
                            tail(0, prev[0] - 3)
                    prev = (b, e_b)
                phase_b(prev[0], prev[1])
                if tail is not None:
                    for piece in range(4):
                        tail(1, piece)

            # ---------- output projection + residual (fp8 DoubleRow) ----------
            def out_project_staged(wdram, layer):
                wsb = []
                for g in range(2):
                    wt = wst.tile([128, KT, 512], F8, tag="wstage", name="wstage")
                    nc.sync.dma_start(out=wt, in_=wdram[layer, g])
                    wsb.append(wt)
                return wsb

            def out_project_chunk(wsb, src, odesc, c, piece):
                sl = slice(c * CH, (c + 1) * CH)
                for m in range(2 * piece, 2 * piece + 2):
                    pp = psmm.tile([128, 512], F32, tag="mm", name="mm")[:, :CH]
                    for k in range(0, KT, 2):
                        nc.tensor.matmul(pp,
                                         lhsT=wsb[m // 4][:, k:k + 2, (m % 4) * 128:(m % 4 + 1) * 128],
                                         rhs=src[:, k:k + 2, sl],
                                         start=(k == 0), stop=(k == KT - 2),
                                         perf_mode=DRMODE)
                    nc.vector.scalar_tensor_tensor(out=xT[:, m, sl], in0=pp,
                                                   scalar=odesc, in1=xT[:, m, sl],
                                                   op0=ALU.mult, op1=ALU.add)

            # ---------- FFN (weights staged once; hT holds both chunks) ----------
            def ffn(layer, xn):
                gelu_f = AF.Gelu if gelu_exact else AF.Square
                hT = big.tile([128, FF // 128, TOK], BF, tag="obuf", name="obuf")
                for g in range(FF // 512):
                    wt = wst.tile([128, KT, 512], BF, tag="wstage", name="wstage")
                    nc.sync.dma_start(out=wt, in_=w1[layer, g])
                    for c in range(NCH):
                        sl = slice(c * CH, (c + 1) * CH)
                        for mm in range(4):
                            fm = 4 * g + mm
                            hp = psmm.tile([128, 512], F32, tag="mm", name="mm")[:, :CH]
                            for k in range(KT):
                                nc.tensor.matmul(hp, lhsT=wt[:, k, mm * 128:(mm + 1) * 128],
                                                 rhs=xn[:, k, sl],
                                                 start=(k == 0), stop=(k == KT - 1))
                            nc.scalar.activation(out=hT[:, fm, sl], in_=hp, func=gelu_f,
                                                 bias=b1_sb[:, layer, fm, None], scale=1.0)
                for qm in range(4):  # 256-wide output blocks, both chunks at once
                    yps = [psmm.tile([128, 512], F32, tag="mm", name="mm")[:, :CH]
                           for _ in range(4)]  # index 2*c + mm
                    for kg in range(4):
                        wt = wst.tile([128, KT, 256], BF, tag="wstage2", name="wstage2")
                        nc.sync.dma_start(out=wt, in_=w2[layer, kg, qm])
                        for c in range(NCH):
                            sl = slice(c * CH, (c + 1) * CH)
                            for k in range(KT):
                                for mm in range(2):
                                    nc.tensor.matmul(
                                        yps[2 * c + mm],
                                        lhsT=wt[:, k, mm * 128:(mm + 1) * 128],
                                        rhs=hT[:, kg * KT + k, sl],
                                        start=(kg == 0 and k == 0),
                                        stop=(kg == 3 and k == KT - 1))
                    for c in range(NCH):
                        sl = slice(c * CH, (c + 1) * CH)
                        for mm in range(2):
                            nc.vector.tensor_tensor(out=xT[:, 2 * qm + mm, sl],
                                                    in0=yps[2 * c + mm],
                                                    in1=xT[:, 2 * qm + mm, sl],
                                                    op=ALU.add)

            # ================= main =================
            for layer in range(DEPTH):
                xn = big.tile([128, KT, TOK], F8, tag="xn", name="xn")
                rmsnorm(xn, sx=SXA)
                qT = big.tile([128, KT, TOK], F8, tag="qbuf", name="qbuf")
                kT = big.tile([128, KT, TOK], F8, tag="kbuf", name="kbuf")
                vv = big.tile([128, BB, INNER], F8, tag="vbuf", name="vbuf")
                o_all = big.tile([128, KT, TOK], F8, tag="obuf", name="obuf")
                qk_project(qT, wq, layer, xn, TOK, cosq, sinq, crs=cq_a)
                qk_project(kT, wk, layer, xn, TOK, cosq, sinq, crs=ck_a)
                v_project(vv, wv, layer, xn, TOK, cross=False, vdesc=vdesc_a)
                wsb_o = out_project_staged(wo, layer)
                attention(qT, kT, vv, o_all, 128, cross=False,
                          tail=lambda c, p: out_project_chunk(wsb_o, o_all, odesc_a, c, p))

                xn = big.tile([128, KT, TOK], F8, tag="xn", name="xn")
                rmsnorm(xn, sx=SXA)
                qT = big.tile([128, KT, TOK], F8, tag="qbuf", name="qbuf")
                kTc = big.tile([128, KT, TOKC], F8, tag="kbuf", name="kbuf")
                vvc = big.tile([128, BB, INNER], F8, tag="vbuf", name="vbuf")
                o_all = big.tile([128, KT, TOK], F8, tag="obuf", name="obuf")
                qk_project(qT, wqc, layer, xn, TOK, cosq, sinq, crs=cq_c)
                qk_project(kTc, wkc, layer, csT, TOKC, cosc, sinc, crs=ck_c)
                v_project(vvc, wvc, layer, csT, TOKC, cross=True, vdesc=vdesc_c)
                wsb_oc = out_project_staged(woc, layer)
                attention(qT, kTc, vvc, o_all, M_CTX, cross=True,
                          tail=lambda c, p: out_project_chunk(wsb_oc, o_all, odesc_c, c, p))

                xn = big.tile([128, KT, TOK], BF, tag="xn", name="xn")
                rmsnorm(xn)
                ffn(layer, xn)

            rmsnorm(None, dst_f32=outT, gcol=None if gout_ones else gout_sb)

    nc.compile()
    return nc


_NC_CACHE = {}


def _get_nc(BB, DEPTH, scales, gelu_exact=True, has_bias=False):
    key = (BB, DEPTH, scales, gelu_exact, has_bias)
    if key not in _NC_CACHE:
        _NC_CACHE[key] = _build(BB, DEPTH, scales, gelu_exact, has_bias)
    return _NC_CACHE[key]


def _fp8_scale(w):
    """Largest power-of-2 scale keeping |w*s| <= 192 (e4m3 max finite 240)."""
    mx = float(np.abs(w).max())
    return float(2.0 ** np.floor(np.log2(192.0 / max(mx, 1e-30))))


def _stage2d(W, nblk):
    """[L, K, M] -> [L, nblk, 128, K//128, 512] staging layout (contiguous per
    partition per block, so each staging DMA is 128 linear descriptors)."""
    L, K, M = W.shape
    kt = K // 128
    assert M == nblk * 512
    return np.ascontiguousarray(
        W.reshape(L, kt, 128, nblk, 512).transpose(0, 3, 2, 1, 4))


def _stage_w2(W2):
    """[L, FF, DIM] -> [L, 4 kg, 4 qm, 128, KT, 256]."""
    L = W2.shape[0]
    return np.ascontiguousarray(
        W2.reshape(L, 4, KT, 128, 4, 256).transpose(0, 1, 4, 3, 2, 5))


def _to_fp8(w, s):
    q = (np.asarray(w, np.float32) * s).astype(E4)
    assert np.isfinite(q.astype(np.float32)).all()
    return q


def _rope_tables(n_pos, n_cols):
    """Masked full-head tables [128, n_cols]: rope rows (d%64<32) carry cos/sin,
    pass rows carry cos=1, sin=0. Token columns are batch-periodic."""
    inv = 1.0 / (10000.0 ** (np.arange(0, ROT, 2, dtype=np.float32) / ROT))  # [16]
    pos = np.arange(n_cols, dtype=np.float32) % n_pos
    d = np.arange(64)
    f = inv[d % 16]
    ang = f[:, None] * pos[None, :]
    cos = np.cos(ang)
    sin = np.sin(ang) * np.where(d % 32 < 16, -1.0, 1.0).astype(np.float32)[:, None]
    mask_rope = (d < 32)[:, None]
    cos = np.where(mask_rope, cos, 1.0)
    sin = np.where(mask_rope, sin, 0.0)
    return (np.tile(cos, (2, 1)).astype(BF16), np.tile(sin, (2, 1)).astype(BF16))


def _pack_qk(W):
    return W  # natural layout; rotation happens on-device via the shift matmul


def _shift_matrix():
    """S [128,128] bf16: out[m] = in[src(m)] for rope rows, 0 for pass rows."""
    S = np.zeros((128, 128), np.float32)
    for m in range(128):
        d = m % 64
        if d < 32:
            S[64 * (m // 64) + (d + 16) % 32, m] = 1.0
    return S.astype(BF16)


def _prep_weights(inputs, DEPTH):
    f32 = np.float32
    g_attn = np.asarray(inputs["g_attn"], f32)
    g_cross = np.asarray(inputs["g_cross"], f32)
    g_ff = np.asarray(inputs["g_ff"], f32)
    out = {}
    wq_l, wk_l, wv_l, wqc_l, wkc_l, wvc_l = [], [], [], [], [], []
    for L in range(DEPTH):
        Wq = np.asarray(inputs["Wq_a"][L], f32) * g_attn[L][:, None] * (DH ** -0.5)
        Wkv = np.asarray(inputs["Wkv_a"][L], f32) * g_attn[L][:, None]
        wq_l.append(_pack_qk(Wq))
        wk_l.append(_pack_qk(Wkv[:, :INNER]))
        wv_l.append(Wkv[:, INNER:])
        Wqc = np.asarray(inputs["Wq_c"][L], f32) * g_cross[L][:, None] * (DH ** -0.5)
        Wkvc = np.asarray(inputs["Wkv_c"][L], f32)  # context is not normed
        wqc_l.append(_pack_qk(Wqc))
        wkc_l.append(_pack_qk(Wkvc[:, :INNER]))
        wvc_l.append(Wkvc[:, INNER:])
    wo_f = np.asarray(inputs["Wo_a"], f32)[:DEPTH]
    woc_f = np.asarray(inputs["Wo_c"], f32)[:DEPTH]
    stacks = dict(wq=np.stack(wq_l), wk=np.stack(wk_l), wv=np.stack(wv_l),
                  wo=wo_f, wqc=np.stack(wqc_l), wkc=np.stack(wkc_l),
                  wvc=np.stack(wvc_l), woc=woc_f)
    scales = tuple(_fp8_scale(stacks[n]) for n in
                   ("wq", "wk", "wv", "wo", "wqc", "wkc", "wvc", "woc"))
    scales = scales + (bool(np.all(np.asarray(inputs["g_out"], f32) == 1.0)),)
    for n, s in zip(("wq", "wk", "wv", "wo", "wqc", "wkc", "wvc", "woc"), scales[:8]):
        out[n] = _stage2d(_to_fp8(stacks[n], s), 2)
    out["w1"] = _stage2d(
        (np.asarray(inputs["W1"], f32)[:DEPTH] * g_ff[:DEPTH, :, None]).astype(BF16), 8)
    out["w2"] = _stage_w2(np.asarray(inputs["W2"], f32)[:DEPTH].astype(BF16))
    out["b1c"] = np.ascontiguousarray(
        np.asarray(inputs["b1"], f32)[:DEPTH].reshape(DEPTH, FF // 128, 128).transpose(0, 2, 1))
    out["goutc"] = np.ascontiguousarray(np.asarray(inputs["g_out"], f32).reshape(KT, 128).T)
    brows = np.stack([np.asarray(inputs["bo_a"], f32)[:DEPTH],
                      np.asarray(inputs["bo_c"], f32)[:DEPTH],
                      np.asarray(inputs["b2"], f32)[:DEPTH]], axis=1)
    has_bias = bool(np.any(brows))
    assert not has_bias, "fp8 out_project path dropped the bias matmul"
    return out, has_bias, scales


def prepare(inputs, BB, DEPTH, n_cores):
    """Returns (in_maps, has_bias, scales) for n_cores cores."""
    TOK, TOKC = BB * N_CTX, BB * M_CTX
    CH, CHC = min(512, TOK), min(512, TOKC)
    w, has_bias, scales = _prep_weights(inputs, DEPTH)
    cosq, sinq = _rope_tables(N_CTX, CH)
    cosc, sinc = _rope_tables(M_CTX, CHC)
    x = np.asarray(inputs["x"], np.float32)
    cs = np.asarray(inputs["chunked_seq"], np.float32)
    in_maps = []
    for c in range(n_cores):
        xs = x[c * BB:(c + 1) * BB]
        css = cs[c * BB:(c + 1) * BB]
        m = dict(w)
        m["xT"] = np.ascontiguousarray(xs.reshape(TOK, DIM).T)
        m["csT"] = _to_fp8(np.ascontiguousarray(css.reshape(TOKC, DIM).T), SXC)
        m["cosq"], m["sinq"] = cosq, sinq
        m["shiftm"] = _shift_matrix()
        m["cosc"], m["sinc"] = cosc, sinc
        in_maps.append(m)
    return in_maps, has_bias, scales


def run_cores(inputs, BB, DEPTH, n_cores, gelu_exact=True):
    in_maps, has_bias, scales = prepare(inputs, BB, DEPTH, n_cores)
    nc = _get_nc(BB, DEPTH, scales, gelu_exact, has_bias)
    res = run_bass_kernel_spmd(nc, in_maps, list(range(n_cores)))
    outs = []
    for c in range(n_cores):
        oT = res.results[c]["outT"]
        outs.append(np.asarray(oT, np.float32).T.reshape(BB, N_CTX, DIM))
    return np.concatenate(outs, axis=0)


def kernel(**inputs):
    return run_cores(inputs, BB=8, DEPTH=3, n_cores=N_CORES).astype(np.float32)



# revision 92
# speedup vs baseline: 1.0020x; 1.0005x over previous
"""Trainium2 Bass kernel for a 3-layer transformer encoder (self+cross attention, FFN).

Data-parallel over batch: 64 batches split as 8 per NeuronCore.
Residual stream kept feature-major fp32 [DIM partitions, tokens].

Precision/perf strategy:
- All attention projections (Q/K/V/O, self+cross) run as fp8-e4m3 DoubleRow
  matmuls (two k-tiles contracted per instruction at 0.5 cycles/row), with
  power-of-2 per-tensor weight scales computed on the host and descale factors
  folded into existing activation copies / the softmax exp / the residual add.
- The FFN stays bf16 (fp8 there costs ~2e-2 rel err); its weights are staged
  once per layer in a contiguous per-partition dram layout, with hT holding
  both 512-token chunks so W2 streams only once.
- Attention computes sim transposed ([k, q] per head) so the fp8 exp output
  feeds attn@v directly (no transposes / psum round trips); softmax
  denominators come from an all-ones-stationary PE matmul broadcast across
  partitions, and the normalization is a reciprocal-multiply on the [dh, q]
  attention output. The out-projection is interleaved into the attention batch
  loop in 2-m-tile pieces to fill PE bubbles.
- RoPE: rotate-half is one PE matmul against a constant shift matrix; the two
  elementwise multiplies are split across Pool (cos, all-SBUF bf16) and DVE
  (sin, psum operand); rmsnorm statistics use fp8-DR matmuls and a
  recip+Sqrt tail (avoids the Ln/Exp act-table reload thrash).
"""

import numpy as np
import ml_dtypes

import concourse.mybir as mybir
import concourse.tile as tile
from concourse import bacc
from concourse.bass_utils import run_bass_kernel_spmd
from concourse.masks import make_identity

BF16 = ml_dtypes.bfloat16
E4 = ml_dtypes.float8_e4m3
F32 = mybir.dt.float32
BF = mybir.dt.bfloat16
F8 = mybir.dt.float8e4
DRMODE = mybir.MatmulPerfMode.DoubleRow
AF = mybir.ActivationFunctionType
ALU = mybir.AluOpType
AX = mybir.AxisListType

SXA = 32.0   # fp8 scale for rmsnormed activations
SXC = 32.0   # fp8 scale for raw chunked_seq
SO = 16.0    # fp8 scale for attention output (pre out-projection)
S8Q = 256.0  # fp8 scale for rope'd q
S8K = 16.0   # fp8 scale for rope'd k

DIM = 1024
HEADS = 16
DH = 64
INNER = 1024
FF = 4096
ROT = 32
N_CTX = 128   # query tokens per batch
M_CTX = 64    # chunk tokens per batch
N_CORES = 8
KT = DIM // 128  # 8 k-subtiles for DIM contraction


def _build(BB, DEPTH, scales, gelu_exact=True, has_bias=False):
    """Build + compile the per-core bass program for BB local batches, DEPTH layers."""
    TOK = BB * N_CTX
    TOKC = BB * M_CTX
    CH = min(512, TOK)
    NCH = TOK // CH
    CHC = min(512, TOKC)

    swq, swk, swv, swo, swqc, swkc, swvc, swoc, gout_ones = scales
    # rope outputs are rescaled into fp8 range via the STT scalar:
    cq_a = S8Q / (SXA * swq)
    ck_a = S8K / (SXA * swk)
    cq_c = S8Q / (SXA * swqc)
    ck_c = S8K / (SXC * swkc)
    exp_s = 1.0 / (S8Q * S8K)    # same for self and cross
    vdesc_a = SO / (SXA * swv)   # vv carries x16 so o_all = e@vv/d lands at xSO
    vdesc_c = SO / (SXC * swvc)
    odesc_a = 1.0 / (SO * swo)
    odesc_c = 1.0 / (SO * swoc)

    nc = bacc.Bacc("TRN2", target_bir_lowering=False, debug=False)

    xT_in = nc.dram_tensor("xT", [DIM, TOK], F32, kind="ExternalInput")
    csT_in = nc.dram_tensor("csT", [DIM, TOKC], F8, kind="ExternalInput")
    # staged-weight dram layout: [..., block, 128 partitions, KT, 512] so each
    # staging DMA reads one contiguous row per partition (128 descriptors).
    wq = nc.dram_tensor("wq", [DEPTH, 2, 128, KT, 512], F8, kind="ExternalInput")
    wk = nc.dram_tensor("wk", [DEPTH, 2, 128, KT, 512], F8, kind="ExternalInput")
    wv = nc.dram_tensor("wv", [DEPTH, 2, 128, KT, 512], F8, kind="ExternalInput")
    wo = nc.dram_tensor("wo", [DEPTH, 2, 128, KT, 512], F8, kind="ExternalInput")
    wqc = nc.dram_tensor("wqc", [DEPTH, 2, 128, KT, 512], F8, kind="ExternalInput")
    wkc = nc.dram_tensor("wkc", [DEPTH, 2, 128, KT, 512], F8, kind="ExternalInput")
    wvc = nc.dram_tensor("wvc", [DEPTH, 2, 128, KT, 512], F8, kind="ExternalInput")
    woc = nc.dram_tensor("woc", [DEPTH, 2, 128, KT, 512], F8, kind="ExternalInput")
    w1 = nc.dram_tensor("w1", [DEPTH, 8, 128, KT, 512], BF, kind="ExternalInput")
    w2 = nc.dram_tensor("w2", [DEPTH, 4, 4, 128, KT, 256], BF, kind="ExternalInput")
    b1c = nc.dram_tensor("b1c", [DEPTH, 128, FF // 128], F32, kind="ExternalInput")
    goutc = nc.dram_tensor("goutc", [128, KT], F32, kind="ExternalInput")
    shiftm_in = nc.dram_tensor("shiftm", [128, 128], BF, kind="ExternalInput")
    if has_bias:
        brows = nc.dram_tensor("brows", [DEPTH, 3, DIM], BF, kind="ExternalInput")
    cosq_in = nc.dram_tensor("cosq", [128, CH], BF, kind="ExternalInput")
    sinq_in = nc.dram_tensor("sinq", [128, CH], BF, kind="ExternalInput")
    cosc_in = nc.dram_tensor("cosc", [128, CHC], BF, kind="ExternalInput")
    sinc_in = nc.dram_tensor("sinc", [128, CHC], BF, kind="ExternalInput")

    outT = nc.dram_tensor("outT", [DIM, TOK], F32, kind="ExternalOutput")

    with tile.TileContext(nc) as tc:
        with tc.tile_pool(name="singles", bufs=1) as singles, \
             tc.tile_pool(name="big", bufs=1) as big, \
             tc.tile_pool(name="wst", bufs=3) as wst, \
             tc.tile_pool(name="scratch", bufs=2) as scratch, \
             tc.tile_pool(name="qrawp", bufs=1) as qrawp, \
             tc.tile_pool(name="attn", bufs=2) as attn_pool, \
             tc.tile_pool(name="psmm", bufs=5, space="PSUM") as psmm, \
             tc.tile_pool(name="psat", bufs=1, space="PSUM") as psat, \
             tc.tile_pool(name="psnrm", bufs=1, space="PSUM") as psnrm:

            # ---------- constants ----------
            ones_col = singles.tile([128, 1], BF)
            nc.vector.memset(ones_col, 1.0)
            ones8w = singles.tile([128, 2, 64], F8)
            nc.vector.memset(ones8w, 1.0)
            ones_mat = singles.tile([128, 128], BF)
            nc.vector.memset(ones_mat, 1.0)
            ones_row_b = singles.tile([1, CH], BF)
            nc.vector.memset(ones_row_b, 1.0)
            cosq = singles.tile([128, CH], BF)
            nc.sync.dma_start(out=cosq, in_=cosq_in[:])
            sinq = singles.tile([128, CH], BF)
            nc.sync.dma_start(out=sinq, in_=sinq_in[:])
            cosc = singles.tile([128, CHC], BF)
            nc.sync.dma_start(out=cosc, in_=cosc_in[:])
            sinc = singles.tile([128, CHC], BF)
            nc.sync.dma_start(out=sinc, in_=sinc_in[:])
            b1_sb = singles.tile([128, DEPTH, FF // 128], F32)
            nc.sync.dma_start(out=b1_sb, in_=b1c.rearrange("l p m -> p l m"))
            gout_sb = singles.tile([128, KT], F32)
            nc.sync.dma_start(out=gout_sb, in_=goutc[:])
            shiftm = singles.tile([128, 128], BF)
            nc.sync.dma_start(out=shiftm, in_=shiftm_in[:])
            if has_bias:
                brows_sb = singles.tile([1, DEPTH, 3, DIM], BF)
                nc.sync.dma_start(out=brows_sb, in_=brows[None])

            xT = singles.tile([128, KT, TOK], F32)
            in_engines = [nc.sync, nc.scalar, nc.gpsimd]
            for k in range(KT):
                in_engines[k % 3].dma_start(
                    out=xT[:, k],
                    in_=xT_in.rearrange("(kt p) t -> p kt t", p=128)[:, k])
            csT = singles.tile([128, KT, TOKC], F8)
            nc.scalar.dma_start(out=csT, in_=csT_in.rearrange("(kt p) t -> p kt t", p=128))

            # ---------- rmsnorm ----------
            def rmsnorm(dst, dst_f32=None, gcol=None, sx=1.0):
                sq = big.tile([128, KT, TOK], F8, tag="obuf", name="sq")  # obuf slot is dead here
                for c in range(NCH):
                    sl = slice(c * CH, (c + 1) * CH)
                    for k in range(KT):
                        if k % 3 == 2:  # ACT is the boundary bottleneck: 2 ops
                            nc.scalar.activation(out=sq[:, k, sl], in_=xT[:, k, sl],
                                                 func=AF.Square)
                        elif k % 3 == 1:
                            nc.vector.tensor_tensor(out=sq[:, k, sl], in0=xT[:, k, sl],
                                                    in1=xT[:, k, sl], op=ALU.mult)
                        else:
                            nc.gpsimd.tensor_tensor(out=sq[:, k, sl], in0=xT[:, k, sl],
                                                    in1=xT[:, k, sl], op=ALU.mult)
                    sp = psnrm.tile([64, CH], F32, tag="nrm", name="nrm")
                    for k in range(0, KT, 2):
                        nc.tensor.matmul(sp, lhsT=ones8w, rhs=sq[:, k:k + 2, sl],
                                         start=(k == 0), stop=(k == KT - 2),
                                         perf_mode=DRMODE)
                    # rstd = sx/sqrt(ms) via DVE recip + ACT Sqrt (Sqrt/Copy/
                    # Square share act tables with everything, unlike Ln/Exp
                    # whose alternation forced a table reload per chunk)
                    rr = scratch.tile([1, CH], BF, tag="rstd", name="rr")
                    with nc.allow_low_precision(reason="bf16 rmsnorm recip"):
                        nc.vector.reciprocal(out=rr, in_=sp[0:1])
                    rstd_b = scratch.tile([1, CH], BF, tag="rstdb", name="rstd_b")
                    nc.scalar.activation(out=rstd_b, in_=rr, func=AF.Sqrt,
                                         scale=DIM * sx * sx, bias=0.0)
                    bp = psnrm.tile([128, CH], F32, tag="nrm", name="nrm")
                    nc.tensor.matmul(bp, lhsT=ones_row_b[:, :128], rhs=rstd_b,
                                     start=True, stop=True)
                    bp_sb = scratch.tile([128, CH], BF, tag="bpsb", name="bp_sb")
                    nc.scalar.copy(out=bp_sb, in_=bp)
                    for k in range(KT):
                        if dst is not None:
                            eng = nc.vector if k % 4 == 0 else nc.gpsimd
                            eng.tensor_tensor(out=dst[:, k, sl], in0=xT[:, k, sl],
                                              in1=bp_sb, op=ALU.mult)
                        if dst_f32 is not None:
                            of = scratch.tile([128, CH], F32, tag="outf", name="outf")
                            if gcol is None:  # g_out == 1: plain normalize
                                eng2 = nc.vector if k % 4 == 0 else nc.gpsimd
                                eng2.tensor_tensor(out=of, in0=xT[:, k, sl],
                                                   in1=bp_sb, op=ALU.mult)
                            else:
                                nc.vector.scalar_tensor_tensor(
                                    out=of, in0=xT[:, k, sl], scalar=gcol[:, k, None],
                                    in1=bp_sb, op0=ALU.mult, op1=ALU.mult)
                            eng = nc.sync if k % 2 == 0 else nc.scalar
                            eng.dma_start(out=dst_f32[k * 128:(k + 1) * 128, sl],
                                          in_=of)

            # ---------- rope-packed q/k projection (fp8 DoubleRow) ----------
            def qk_project(dst, wdram, layer, src, n_src_tok, cos_t, sin_t, crs):
                ch = min(512, n_src_tok)
                nch = n_src_tok // ch
                for sg in range(2):  # m-tiles 4*sg .. 4*sg+3
                    wn = wst.tile([128, KT, 512], F8, tag="wstage", name="wstage")
                    nc.sync.dma_start(out=wn, in_=wdram[layer, sg])
                    for c in range(nch):
                        sl = slice(c * ch, (c + 1) * ch)
                        qps, qraws = [], []
                        for m in range(4):
                            qp = psmm.tile([128, 512], F32, tag="mm", name="mm")[:, :ch]
                            for k in range(0, KT, 2):
                                nc.tensor.matmul(qp, lhsT=wn[:, k:k + 2, m * 128:(m + 1) * 128],
                                                 rhs=src[:, k:k + 2, sl],
                                                 start=(k == 0), stop=(k == KT - 2),
                                                 perf_mode=DRMODE)
                            # the fp8-range rescale rides the psum->sbuf copy
                            qraw = qrawp.tile([128, 512], BF, tag=f"qraw{m}",
                                                name=f"qraw{m}")[:, :ch]
                            nc.scalar.activation(out=qraw, in_=qp, func=AF.Copy,
                                                 scale=crs)
                            qps.append(qp); qraws.append(qraw)
                        for m in range(4):
                            rp = psat.tile([128, 512], F32, tag="op", name="op")[:, :ch]
                            nc.tensor.matmul(rp, lhsT=shiftm, rhs=qraws[m],
                                             start=True, stop=True)
                            # qraw·cos on Pool (all-SBUF), rp·sin on DVE
                            tcos = scratch.tile([128, 512], BF, tag="tcos", name="tcos")[:, :ch]
                            nc.gpsimd.tensor_tensor(out=tcos, in0=qraws[m],
                                                    in1=cos_t[:, :ch], op=ALU.mult)
                            tsin = scratch.tile([128, 512], BF, tag="tsin", name="tsin")[:, :ch]
                            nc.vector.tensor_tensor(out=tsin, in0=rp,
                                                    in1=sin_t[:, :ch], op=ALU.mult)
                            nc.gpsimd.tensor_tensor(out=dst[:, 4 * sg + m, sl],
                                                    in0=tcos, in1=tsin, op=ALU.add)

            # ---------- v projection (token-major, fp8 DoubleRow) ----------
            def v_project(dst, wdram, layer, src, n_src_tok, cross, vdesc):
                wsb = []
                for g in range(2):
                    wt = wst.tile([128, KT, 512], F8, tag="wstage", name="wstage")
                    nc.sync.dma_start(out=wt, in_=wdram[layer, g])
                    wsb.append(wt)
                for mt in range(n_src_tok // 128):
                    for g in range(2):
                        vp = psmm.tile([128, 512], F32, tag="mm", name="mm")
                        for k in range(0, KT, 2):
                            nc.tensor.matmul(vp, lhsT=src[:, k:k + 2, mt * 128:(mt + 1) * 128],
                                             rhs=wsb[g][:, k:k + 2, :],
                                             start=(k == 0), stop=(k == KT - 2),
                                             perf_mode=DRMODE)
                        if not cross:
                            nc.scalar.activation(out=dst[:, mt, g * 512:(g + 1) * 512],
                                                 in_=vp, func=AF.Copy, scale=vdesc)
                        else:
                            nc.scalar.activation(out=dst[0:64, 2 * mt, g * 512:(g + 1) * 512],
                                                 in_=vp[0:64], func=AF.Copy, scale=vdesc)
                            nc.scalar.activation(out=dst[64:128, 2 * mt + 1, g * 512:(g + 1) * 512],
                                                 in_=vp[64:128], func=AF.Copy, scale=vdesc)

            # ---------- attention core ([k,q]-layout, fp8 e, PE denominators) ----------
            def attention(qT, kT, vv, o_all, n_k, cross, tail=None):
                # sim is computed transposed (out [k, q] per head) so the exp
                # output feeds attn@v directly; softmax denominators come from
                # a ones-matmul over the k partitions; the divide happens on
                # the [dh, q] attention output (16x fewer elements than e).
                order = [2 * i for i in range(HEADS // 2)] + [2 * i + 1 for i in range(HEADS // 2)]
                pos = {h: i for i, h in enumerate(order)}

                def phase_a(b):
                    base = 64 * (b % 2) if cross else 0
                    qsl = slice(b * 128, (b + 1) * 128)
                    ksl = slice(b * n_k, (b + 1) * n_k)
                    e_b = attn_pool.tile([128, HEADS * 128], F8, tag="e_b", name="e_b")
                    for g in range(4):  # 4 heads per psum tile
                        heads = order[g * 4:(g + 1) * 4]
                        sp = psmm.tile([128, 512], F32, tag="mm", name="mm")
                        for j, h in enumerate(heads):
                            hb = 64 * (h % 2)
                            nc.tensor.matmul(
                                sp[base:base + n_k, j * 128:(j + 1) * 128],
                                lhsT=kT[hb:hb + 64, h // 2, ksl],
                                rhs=qT[hb:hb + 64, h // 2, qsl],
                                start=(j == 0), stop=(j == 3),
                                skip_group_check=True)
                        nc.scalar.activation(out=e_b[base:base + n_k, g * 512:(g + 1) * 512],
                                             in_=sp[base:base + n_k], func=AF.Exp,
                                             scale=exp_s)
                    return e_b

                def phase_b(b, e_b):
                    base = 64 * (b % 2) if cross else 0
                    qsl = slice(b * 128, (b + 1) * 128)
                    for half in range(2):
                        # denominators for this half's 8 heads, broadcast to all
                        # 64 partitions by an all-ones stationary: evens on top
                        # partitions, odds on bottom (matches op_ layout)
                        dD = psat.tile([128, 512], F32, tag="dD", name="dD")
                        for par in range(2):
                            esl = slice(par * 1024 + half * 512,
                                        par * 1024 + half * 512 + 512)
                            nc.tensor.matmul(dD[64 * par:64 * par + 64],
                                             lhsT=ones_mat[base:base + n_k, :64],
                                             rhs=e_b[base:base + n_k, esl],
                                             start=True, stop=True,
                                             skip_group_check=True)
                        dr = attn_pool.tile([128, 512], BF, tag="dr", name="dr")
                        with nc.allow_low_precision(reason="bf16 softmax denom"):
                            nc.vector.reciprocal(out=dr, in_=dD)
                        op_ = psat.tile([128, 512], F32, tag="op", name="op")
                        for hp in range(4):
                            h0 = 8 * half + 2 * hp
                            nc.tensor.matmul(
                                op_[0:64, hp * 128:(hp + 1) * 128],
                                lhsT=vv[base:base + n_k, b, h0 * 64:(h0 + 1) * 64],
                                rhs=e_b[base:base + n_k, pos[h0] * 128:(pos[h0] + 1) * 128],
                                start=(hp == 0), stop=False, tile_position=(base, 0),
                                skip_group_check=True)
                            nc.tensor.matmul(
                                op_[64:128, hp * 128:(hp + 1) * 128],
                                lhsT=vv[base:base + n_k, b, (h0 + 1) * 64:(h0 + 2) * 64],
                                rhs=e_b[base:base + n_k, pos[h0 + 1] * 128:(pos[h0 + 1] + 1) * 128],
                                start=(hp == 0), stop=(hp == 3), tile_position=(base, 64),
                                skip_group_check=True)
                        nc.vector.tensor_tensor(
                            out=o_all[:, 4 * half:4 * half + 4, qsl],
                            in0=op_.rearrange("p (hp t) -> p hp t", hp=4),
                            in1=dr.rearrange("p (hp t) -> p hp t", hp=4),
                            op=ALU.mult)

                prev = None
                for b in range(BB):
                    e_b = phase_a(b)
                    if prev is not None:
                        phase_b(prev[0], prev[1])
                        # once batches 0..3 (= token chunk 0) are done, spread
                        # the chunk-0 out-projection pieces over the remaining
                        # batch iterations to fill PE bubbles
                        if prev[0] >= 3 and tail is not None:
                            tail(0, prev[0] - 3)
                    prev = (b, e_b)
                phase_b(prev[0], prev[1])
                if tail is not None:
                    for piece in range(4):
                        tail(1, piece)

            # ---------- output projection + residual (fp8 DoubleRow) ----------
            def out_project_staged(wdram, layer):
                wsb = []
                for g in range(2):
                    wt = wst.tile([128, KT, 512], F8, tag="wstage", name="wstage")
                    nc.sync.dma_start(out=wt, in_=wdram[layer, g])
                    wsb.append(wt)
                return wsb

            def out_project_chunk(wsb, src, odesc, c, piece):
                sl = slice(c * CH, (c + 1) * CH)
                for m in range(2 * piece, 2 * piece + 2):
                    pp = psmm.tile([128, 512], F32, tag="mm", name="mm")[:, :CH]
                    for k in range(0, KT, 2):
                        nc.tensor.matmul(pp,
                                         lhsT=wsb[m // 4][:, k:k + 2, (m % 4) * 128:(m % 4 + 1) * 128],
                                         rhs=src[:, k:k + 2, sl],
                                         start=(k == 0), stop=(k == KT - 2),
                                         perf_mode=DRMODE)
                    nc.vector.scalar_tensor_tensor(out=xT[:, m, sl], in0=pp,
                                                   scalar=odesc, in1=xT[:, m, sl],
                                                   op0=ALU.mult, op1=ALU.add)

            # ---------- FFN (weights staged once; hT holds both chunks) ----------
            def ffn(layer, xn):
                gelu_f = AF.Gelu if gelu_exact else AF.Square
                hT = big.tile([128, FF // 128, TOK], BF, tag="obuf", name="obuf")
                for g in range(FF // 512):
                    wt = wst.tile([128, KT, 512], BF, tag="wstage", name="wstage")
                    nc.sync.dma_start(out=wt, in_=w1[layer, g])
                    for c in range(NCH):
                        sl = slice(c * CH, (c + 1) * CH)
                        for mm in range(4):
                            fm = 4 * g + mm
                            hp = psmm.tile([128, 512], F32, tag="mm", name="mm")[:, :CH]
                            for k in range(KT):
                                nc.tensor.matmul(hp, lhsT=wt[:, k, mm * 128:(mm + 1) * 128],
                                                 rhs=xn[:, k, sl],
                                                 start=(k == 0), stop=(k == KT - 1))
                            nc.scalar.activation(out=hT[:, fm, sl], in_=hp, func=gelu_f,
                                                 bias=b1_sb[:, layer, fm, None], scale=1.0)
                for qm in range(4):  # 256-wide output blocks, both chunks at once
                    yps = [psmm.tile([128, 512], F32, tag="mm", name="mm")[:, :CH]
                           for _ in range(4)]  # index 2*c + mm
                    for kg in range(4):
                        wt = wst.tile([128, KT, 256], BF, tag="wstage2", name="wstage2")
                        nc.sync.dma_start(out=wt, in_=w2[layer, kg, qm])
                        for c in range(NCH):
                            sl = slice(c * CH, (c + 1) * CH)
                            for k in range(KT):
                                for mm in range(2):
                                    nc.tensor.matmul(
                                        yps[2 * c + mm],
                                        lhsT=wt[:, k, mm * 128:(mm + 1) * 128],
                                        rhs=hT[:, kg * KT + k, sl],
                                        start=(kg == 0 and k == 0),
                                        stop=(kg == 3 and k == KT - 1))
                    for c in range(NCH):
                        sl = slice(c * CH, (c + 1) * CH)
                        for mm in range(2):
                            nc.vector.tensor_tensor(out=xT[:, 2 * qm + mm, sl],
                                                    in0=yps[2 * c + mm],
                                                    in1=xT[:, 2 * qm + mm, sl],
                                                    op=ALU.add)

            # ================= main =================
            for layer in range(DEPTH):
                xn = big.tile([128, KT, TOK], F8, tag="xn", name="xn")
                rmsnorm(xn, sx=SXA)
                qT = big.tile([128, KT, TOK], F8, tag="qbuf", name="qbuf")
                kT = big.tile([128, KT, TOK], F8, tag="kbuf", name="kbuf")
                vv = big.tile([128, BB, INNER], F8, tag="vbuf", name="vbuf")
                o_all = big.tile([128, KT, TOK], F8, tag="obuf", name="obuf")
                qk_project(qT, wq, layer, xn, TOK, cosq, sinq, crs=cq_a)
                qk_project(kT, wk, layer, xn, TOK, cosq, sinq, crs=ck_a)
                v_project(vv, wv, layer, xn, TOK, cross=False, vdesc=vdesc_a)
                wsb_o = out_project_staged(wo, layer)
                attention(qT, kT, vv, o_all, 128, cross=False,
                          tail=lambda c, p: out_project_chunk(wsb_o, o_all, odesc_a, c, p))

                xn = big.tile([128, KT, TOK], F8, tag="xn", name="xn")
                rmsnorm(xn, sx=SXA)
                qT = big.tile([128, KT, TOK], F8, tag="qbuf", name="qbuf")
                kTc = big.tile([128, KT, TOKC], F8, tag="kbuf", name="kbuf")
                vvc = big.tile([128, BB, INNER], F8, tag="vbuf", name="vbuf")
                o_all = big.tile([128, KT, TOK], F8, tag="obuf", name="obuf")
                qk_project(qT, wqc, layer, xn, TOK, cosq, sinq, crs=cq_c)
                qk_project(kTc, wkc, layer, csT, TOKC, cosc, sinc, crs=ck_c)
                v_project(vvc, wvc, layer, csT, TOKC, cross=True, vdesc=vdesc_c)
                wsb_oc = out_project_staged(woc, layer)
                attention(qT, kTc, vvc, o_all, M_CTX, cross=True,
                          tail=lambda c, p: out_project_chunk(wsb_oc, o_all, odesc_c, c, p))

                xn = big.tile([128, KT, TOK], BF, tag="xn", name="xn")
                rmsnorm(xn)
                ffn(layer, xn)

            rmsnorm(None, dst_f32=outT, gcol=None if gout_ones else gout_sb)

    nc.compile()
    return nc


_NC_CACHE = {}


def _get_nc(BB, DEPTH, scales, gelu_exact=True, has_bias=False):
    key = (BB, DEPTH, scales, gelu_exact, has_bias)
    if key not in _NC_CACHE:
        _NC_CACHE[key] = _build(BB, DEPTH, scales, gelu_exact, has_bias)
    return _NC_CACHE[key]


def _fp8_scale(w):
    """Largest power-of-2 scale keeping |w*s| <= 192 (e4m3 max finite 240)."""
    mx = float(np.abs(w).max())
    return float(2.0 ** np.floor(np.log2(192.0 / max(mx, 1e-30))))


def _stage2d(W, nblk):
    """[L, K, M] -> [L, nblk, 128, K//128, 512] staging layout (contiguous per
    partition per block, so each staging DMA is 128 linear descriptors)."""
    L, K, M = W.shape
    kt = K // 128
    assert M == nblk * 512
    return np.ascontiguousarray(
        W.reshape(L, kt, 128, nblk, 512).transpose(0, 3, 2, 1, 4))


def _stage_w2(W2):
    """[L, FF, DIM] -> [L, 4 kg, 4 qm, 128, KT, 256]."""
    L = W2.shape[0]
    return np.ascontiguousarray(
        W2.reshape(L, 4, KT, 128, 4, 256).transpose(0, 1, 4, 3, 2, 5))


def _to_fp8(w, s):
    q = (np.asarray(w, np.float32) * s).astype(E4)
    assert np.isfinite(q.astype(np.float32)).all()
    return q


def _rope_tables(n_pos, n_cols):
    """Masked full-head tables [128, n_cols]: rope rows (d%64<32) carry cos/sin,
    pass rows carry cos=1, sin=0. Token columns are batch-periodic."""
    inv = 1.0 / (10000.0 ** (np.arange(0, ROT, 2, dtype=np.float32) / ROT))  # [16]
    pos = np.arange(n_cols, dtype=np.float32) % n_pos
    d = np.arange(64)
    f = inv[d % 16]
    ang = f[:, None] * pos[None, :]
    cos = np.cos(ang)
    sin = np.sin(ang) * np.where(d % 32 < 16, -1.0, 1.0).astype(np.float32)[:, None]
    mask_rope = (d < 32)[:, None]
    cos = np.where(mask_rope, cos, 1.0)
    sin = np.where(mask_rope, sin, 0.0)
    return (np.tile(cos, (2, 1)).astype(BF16), np.tile(sin, (2, 1)).astype(BF16))


def _pack_qk(W):
    return W  # natural layout; rotation happens on-device via the shift matmul


def _shift_matrix():
    """S [128,128] bf16: out[m] = in[src(m)] for rope rows, 0 for pass rows."""
    S = np.zeros((128, 128), np.float32)
    for m in range(128):
        d = m % 64
        if d < 32:
            S[64 * (m // 64) + (d + 16) % 32, m] = 1.0
    return S.astype(BF16)


def _prep_weights(inputs, DEPTH):
    f32 = np.float32
    g_attn = np.asarray(inputs["g_attn"], f32)
    g_cross = np.asarray(inputs["g_cross"], f32)
    g_ff = np.asarray(inputs["g_ff"], f32)
    out = {}
    wq_l, wk_l, wv_l, wqc_l, wkc_l, wvc_l = [], [], [], [], [], []
    for L in range(DEPTH):
        Wq = np.asarray(inputs["Wq_a"][L], f32) * g_attn[L][:, None] * (DH ** -0.5)
        Wkv = np.asarray(inputs["Wkv_a"][L], f32) * g_attn[L][:, None]
        wq_l.append(_pack_qk(Wq))
        wk_l.append(_pack_qk(Wkv[:, :INNER]))
        wv_l.append(Wkv[:, INNER:])
        Wqc = np.asarray(inputs["Wq_c"][L], f32) * g_cross[L][:, None] * (DH ** -0.5)
        Wkvc = np.asarray(inputs["Wkv_c"][L], f32)  # context is not normed
        wqc_l.append(_pack_qk(Wqc))
        wkc_l.append(_pack_qk(Wkvc[:, :INNER]))
        wvc_l.append(Wkvc[:, INNER:])
    wo_f = np.asarray(inputs["Wo_a"], f32)[:DEPTH]
    woc_f = np.asarray(inputs["Wo_c"], f32)[:DEPTH]
    stacks = dict(wq=np.stack(wq_l), wk=np.stack(wk_l), wv=np.stack(wv_l),
                  wo=wo_f, wqc=np.stack(wqc_l), wkc=np.stack(wkc_l),
                  wvc=np.stack(wvc_l), woc=woc_f)
    scales = tuple(_fp8_scale(stacks[n]) for n in
                   ("wq", "wk", "wv", "wo", "wqc", "wkc", "wvc", "woc"))
    scales = scales + (bool(np.all(np.asarray(inputs["g_out"], f32) == 1.0)),)
    for n, s in zip(("wq", "wk", "wv", "wo", "wqc", "wkc", "wvc", "woc"), scales[:8]):
        out[n] = _stage2d(_to_fp8(stacks[n], s), 2)
    out["w1"] = _stage2d(
        (np.asarray(inputs["W1"], f32)[:DEPTH] * g_ff[:DEPTH, :, None]).astype(BF16), 8)
    out["w2"] = _stage_w2(np.asarray(inputs["W2"], f32)[:DEPTH].astype(BF16))
    out["b1c"] = np.ascontiguousarray(
        np.asarray(inputs["b1"], f32)[:DEPTH].reshape(DEPTH, FF // 128, 128).transpose(0, 2, 1))
    out["goutc"] = np.ascontiguousarray(np.asarray(inputs["g_out"], f32).reshape(KT, 128).T)
    brows = np.stack([np.asarray(inputs["bo_a"], f32)[:DEPTH],
                      np.asarray(inputs["bo_c"], f32)[:DEPTH],
                      np.asarray(inputs["b2"], f32)[:DEPTH]], axis=1)
    has_bias = bool(np.any(brows))
    assert not has_bias, "fp8 out_project path dropped the bias matmul"
    return out, has_bias, scales


def prepare(inputs, BB, DEPTH, n_cores):
    """Returns (in_maps, has_bias, scales) for n_cores cores."""
    TOK, TOKC = BB * N_CTX, BB * M_CTX
    CH, CHC = min(512, TOK), min(512, TOKC)
    w, has_bias, scales = _prep_weights(inputs, DEPTH)
    cosq, sinq = _rope_tables(N_CTX, CH)
    cosc, sinc = _rope_tables(M_CTX, CHC)
    x = np.asarray(inputs["x"], np.float32)
    cs = np.asarray(inputs["chunked_seq"], np.float32)
    in_maps = []
    for c in range(n_cores):
        xs = x[c * BB:(c + 1) * BB]
        css = cs[c * BB:(c + 1) * BB]
        m = dict(w)
        m["xT"] = np.ascontiguousarray(xs.reshape(TOK, DIM).T)
        m["csT"] = _to_fp8(np.ascontiguousarray(css.reshape(TOKC, DIM).T), SXC)
        m["cosq"], m["sinq"] = cosq, sinq
        m["shiftm"] = _shift_matrix()
        m["cosc"], m["sinc"] = cosc, sinc
        in_maps.append(m)
    return in_maps, has_bias, scales


def run_cores(inputs, BB, DEPTH, n_cores, gelu_exact=True):
    in_maps, has_bias, scales = prepare(inputs, BB, DEPTH, n_cores)
    nc = _get_nc(BB, DEPTH, scales, gelu_exact, has_bias)
    res = run_bass_kernel_spmd(nc, in_maps, list(range(n_cores)))
    outs = []
    for c in range(n_cores):
        oT = res.results[c]["outT"]
        outs.append(np.asarray(oT, np.float32).T.reshape(BB, N_CTX, DIM))
    return np.concatenate(outs, axis=0)


def kernel(**inputs):
    return run_cores(inputs, BB=8, DEPTH=3, n_cores=N_CORES).astype(np.float32)



# revision 93
# speedup vs baseline: 1.0106x; 1.0086x over previous
"""Trainium2 Bass kernel for a 3-layer transformer encoder (self+cross attention, FFN).

Data-parallel over batch: 64 batches split as 8 per NeuronCore.
Residual stream kept feature-major fp32 [DIM partitions, tokens].

Precision/perf strategy:
- All attention projections (Q/K/V/O, self+cross) run as fp8-e4m3 DoubleRow
  matmuls (two k-tiles contracted per instruction at 0.5 cycles/row), with
  power-of-2 per-tensor weight scales computed on the host and descale factors
  folded into existing activation copies / the softmax exp / the residual add.
- The FFN stays bf16 (fp8 there costs ~2e-2 rel err); its weights are staged
  once per layer in a contiguous per-partition dram layout, with hT holding
  both 512-token chunks so W2 streams only once.
- Attention computes sim transposed ([k, q] per head) so the fp8 exp output
  feeds attn@v directly (no transposes / psum round trips); softmax
  denominators come from an all-ones-stationary PE matmul broadcast across
  partitions, and the normalization is a reciprocal-multiply on the [dh, q]
  attention output. The out-projection is interleaved into the attention batch
  loop in 2-m-tile pieces to fill PE bubbles.
- RoPE: rotate-half is one PE matmul against a constant shift matrix; the two
  elementwise multiplies are split across Pool (cos, all-SBUF bf16) and DVE
  (sin, psum operand); rmsnorm statistics use fp8-DR matmuls and a
  recip+Sqrt tail (avoids the Ln/Exp act-table reload thrash).
"""

import numpy as np
import ml_dtypes

import concourse.mybir as mybir
import concourse.tile as tile
from concourse import bacc
from concourse.bass_utils import run_bass_kernel_spmd
from concourse.masks import make_identity

BF16 = ml_dtypes.bfloat16
E4 = ml_dtypes.float8_e4m3
F32 = mybir.dt.float32
BF = mybir.dt.bfloat16
F8 = mybir.dt.float8e4
DRMODE = mybir.MatmulPerfMode.DoubleRow
AF = mybir.ActivationFunctionType
ALU = mybir.AluOpType
AX = mybir.AxisListType

SXA = 32.0   # fp8 scale for rmsnormed activations
SXC = 32.0   # fp8 scale for raw chunked_seq
SO = 16.0    # fp8 scale for attention output (pre out-projection)
S8Q = 256.0  # fp8 scale for rope'd q
S8K = 16.0   # fp8 scale for rope'd k

DIM = 1024
HEADS = 16
DH = 64
INNER = 1024
FF = 4096
ROT = 32
N_CTX = 128   # query tokens per batch
M_CTX = 64    # chunk tokens per batch
N_CORES = 8
KT = DIM // 128  # 8 k-subtiles for DIM contraction


def _build(BB, DEPTH, scales, gelu_exact=True, has_bias=False):
    """Build + compile the per-core bass program for BB local batches, DEPTH layers."""
    TOK = BB * N_CTX
    TOKC = BB * M_CTX
    CH = min(512, TOK)
    NCH = TOK // CH
    CHC = min(512, TOKC)

    swq, swk, swv, swo, swqc, swkc, swvc, swoc, gout_ones = scales
    # rope outputs are rescaled into fp8 range via the STT scalar:
    cq_a = S8Q / (SXA * swq)
    ck_a = S8K / (SXA * swk)
    cq_c = S8Q / (SXA * swqc)
    ck_c = S8K / (SXC * swkc)
    exp_s = 1.0 / (S8Q * S8K)    # same for self and cross
    vdesc_a = SO / (SXA * swv)   # vv carries x16 so o_all = e@vv/d lands at xSO
    vdesc_c = SO / (SXC * swvc)
    odesc_a = 1.0 / (SO * swo)
    odesc_c = 1.0 / (SO * swoc)

    nc = bacc.Bacc("TRN2", target_bir_lowering=False, debug=False)

    xT_in = nc.dram_tensor("xT", [DIM, TOK], F32, kind="ExternalInput")
    csT_in = nc.dram_tensor("csT", [DIM, TOKC], F8, kind="ExternalInput")
    # staged-weight dram layout: [..., block, 128 partitions, KT, 512] so each
    # staging DMA reads one contiguous row per partition (128 descriptors).
    wq = nc.dram_tensor("wq", [DEPTH, 2, 128, KT, 512], F8, kind="ExternalInput")
    wk = nc.dram_tensor("wk", [DEPTH, 2, 128, KT, 512], F8, kind="ExternalInput")
    wv = nc.dram_tensor("wv", [DEPTH, 2, 128, KT, 512], F8, kind="ExternalInput")
    wo = nc.dram_tensor("wo", [DEPTH, 2, 128, KT, 512], F8, kind="ExternalInput")
    wqc = nc.dram_tensor("wqc", [DEPTH, 2, 128, KT, 512], F8, kind="ExternalInput")
    wkc = nc.dram_tensor("wkc", [DEPTH, 2, 128, KT, 512], F8, kind="ExternalInput")
    wvc = nc.dram_tensor("wvc", [DEPTH, 2, 128, KT, 512], F8, kind="ExternalInput")
    woc = nc.dram_tensor("woc", [DEPTH, 2, 128, KT, 512], F8, kind="ExternalInput")
    w1 = nc.dram_tensor("w1", [DEPTH, 8, 128, KT, 512], BF, kind="ExternalInput")
    w2 = nc.dram_tensor("w2", [DEPTH, 4, 4, 128, KT, 256], BF, kind="ExternalInput")
    b1c = nc.dram_tensor("b1c", [DEPTH, 128, FF // 128], F32, kind="ExternalInput")
    goutc = nc.dram_tensor("goutc", [128, KT], F32, kind="ExternalInput")
    shiftm_in = nc.dram_tensor("shiftm", [128, 128], BF, kind="ExternalInput")
    if has_bias:
        brows = nc.dram_tensor("brows", [DEPTH, 3, DIM], BF, kind="ExternalInput")
    cosq_in = nc.dram_tensor("cosq", [128, CH], BF, kind="ExternalInput")
    sinq_in = nc.dram_tensor("sinq", [128, CH], BF, kind="ExternalInput")
    cosc_in = nc.dram_tensor("cosc", [128, CHC], BF, kind="ExternalInput")
    sinc_in = nc.dram_tensor("sinc", [128, CHC], BF, kind="ExternalInput")

    outT = nc.dram_tensor("outT", [DIM, TOK], F32, kind="ExternalOutput")

    with tile.TileContext(nc) as tc:
        with tc.tile_pool(name="singles", bufs=1) as singles, \
             tc.tile_pool(name="big", bufs=1) as big, \
             tc.tile_pool(name="wst", bufs=3) as wst, \
             tc.tile_pool(name="scratch", bufs=2) as scratch, \
             tc.tile_pool(name="qrawp", bufs=1) as qrawp, \
             tc.tile_pool(name="attn", bufs=2) as attn_pool, \
             tc.tile_pool(name="psmm", bufs=5, space="PSUM") as psmm, \
             tc.tile_pool(name="psat", bufs=1, space="PSUM") as psat, \
             tc.tile_pool(name="psnrm", bufs=1, space="PSUM") as psnrm:

            # ---------- constants ----------
            ones_col = singles.tile([128, 1], BF)
            nc.vector.memset(ones_col, 1.0)
            ones8w = singles.tile([128, 2, 64], F8)
            nc.vector.memset(ones8w, 1.0)
            ones_mat = singles.tile([128, 128], BF)
            nc.vector.memset(ones_mat, 1.0)
            ones_row_b = singles.tile([1, CH], BF)
            nc.vector.memset(ones_row_b, 1.0)
            cosq = singles.tile([128, CH], BF)
            nc.sync.dma_start(out=cosq, in_=cosq_in[:])
            sinq = singles.tile([128, CH], BF)
            nc.sync.dma_start(out=sinq, in_=sinq_in[:])
            cosc = singles.tile([128, CHC], BF)
            nc.sync.dma_start(out=cosc, in_=cosc_in[:])
            sinc = singles.tile([128, CHC], BF)
            nc.sync.dma_start(out=sinc, in_=sinc_in[:])
            b1_sb = singles.tile([128, DEPTH, FF // 128], F32)
            nc.sync.dma_start(out=b1_sb, in_=b1c.rearrange("l p m -> p l m"))
            gout_sb = singles.tile([128, KT], F32)
            nc.sync.dma_start(out=gout_sb, in_=goutc[:])
            shiftm = singles.tile([128, 128], BF)
            nc.sync.dma_start(out=shiftm, in_=shiftm_in[:])
            if has_bias:
                brows_sb = singles.tile([1, DEPTH, 3, DIM], BF)
                nc.sync.dma_start(out=brows_sb, in_=brows[None])

            xT = singles.tile([128, KT, TOK], F32)
            in_engines = [nc.sync, nc.scalar, nc.gpsimd]
            for k in range(KT):
                in_engines[k % 3].dma_start(
                    out=xT[:, k],
                    in_=xT_in.rearrange("(kt p) t -> p kt t", p=128)[:, k])
            csT = singles.tile([128, KT, TOKC], F8)
            nc.scalar.dma_start(out=csT, in_=csT_in.rearrange("(kt p) t -> p kt t", p=128))

            # ---------- rmsnorm ----------
            def rmsnorm(dst, dst_f32=None, gcol=None, sx=1.0):
                sq = big.tile([128, KT, TOK], F8, tag="obuf", name="sq")  # obuf slot is dead here
                for c in range(NCH):
                    sl = slice(c * CH, (c + 1) * CH)
                    for k in range(KT):
                        if k % 3 == 2:  # ACT is the boundary bottleneck: 2 ops
                            nc.scalar.activation(out=sq[:, k, sl], in_=xT[:, k, sl],
                                                 func=AF.Square)
                        elif k % 3 == 1:
                            nc.vector.tensor_tensor(out=sq[:, k, sl], in0=xT[:, k, sl],
                                                    in1=xT[:, k, sl], op=ALU.mult)
                        else:
                            nc.gpsimd.tensor_tensor(out=sq[:, k, sl], in0=xT[:, k, sl],
                                                    in1=xT[:, k, sl], op=ALU.mult)
                    sp = psnrm.tile([64, CH], F32, tag="nrm", name="nrm")
                    for k in range(0, KT, 2):
                        nc.tensor.matmul(sp, lhsT=ones8w, rhs=sq[:, k:k + 2, sl],
                                         start=(k == 0), stop=(k == KT - 2),
                                         perf_mode=DRMODE)
                    # rstd = sx/sqrt(ms) via DVE recip + ACT Sqrt (Sqrt/Copy/
                    # Square share act tables with everything, unlike Ln/Exp
                    # whose alternation forced a table reload per chunk)
                    rr = scratch.tile([1, CH], BF, tag="rstd", name="rr")
                    with nc.allow_low_precision(reason="bf16 rmsnorm recip"):
                        nc.vector.reciprocal(out=rr, in_=sp[0:1])
                    rstd_b = scratch.tile([1, CH], BF, tag="rstdb", name="rstd_b")
                    nc.scalar.activation(out=rstd_b, in_=rr, func=AF.Sqrt,
                                         scale=DIM * sx * sx, bias=0.0)
                    bp = psnrm.tile([128, CH], F32, tag="nrm", name="nrm")
                    nc.tensor.matmul(bp, lhsT=ones_row_b[:, :128], rhs=rstd_b,
                                     start=True, stop=True)
                    bp_sb = scratch.tile([128, CH], BF, tag="bpsb", name="bp_sb")
                    nc.scalar.copy(out=bp_sb, in_=bp)
                    for k in range(KT):
                        if dst is not None:
                            eng = nc.vector if k % 4 == 0 else nc.gpsimd
                            eng.tensor_tensor(out=dst[:, k, sl], in0=xT[:, k, sl],
                                              in1=bp_sb, op=ALU.mult)
                        if dst_f32 is not None:
                            of = scratch.tile([128, CH], F32, tag="outf", name="outf")
                            if gcol is None:  # g_out == 1: plain normalize
                                eng2 = nc.vector if k % 4 == 0 else nc.gpsimd
                                eng2.tensor_tensor(out=of, in0=xT[:, k, sl],
                                                   in1=bp_sb, op=ALU.mult)
                            else:
                                nc.vector.scalar_tensor_tensor(
                                    out=of, in0=xT[:, k, sl], scalar=gcol[:, k, None],
                                    in1=bp_sb, op0=ALU.mult, op1=ALU.mult)
                            eng = nc.sync if k % 2 == 0 else nc.scalar
                            eng.dma_start(out=dst_f32[k * 128:(k + 1) * 128, sl],
                                          in_=of)

            # ---------- rope-packed q/k projection (fp8 DoubleRow) ----------
            def qk_project(dst, wdram, layer, src, n_src_tok, cos_t, sin_t, crs):
                ch = min(512, n_src_tok)
                nch = n_src_tok // ch
                for sg in range(2):  # m-tiles 4*sg .. 4*sg+3
                    wn = wst.tile([128, KT, 512], F8, tag="wstage", name="wstage")
                    nc.sync.dma_start(out=wn, in_=wdram[layer, sg])
                    for c in range(nch):
                        sl = slice(c * ch, (c + 1) * ch)
                        qps, qraws = [], []
                        for m in range(4):
                            qp = psmm.tile([128, 512], F32, tag="mm", name="mm")[:, :ch]
                            for k in range(0, KT, 2):
                                nc.tensor.matmul(qp, lhsT=wn[:, k:k + 2, m * 128:(m + 1) * 128],
                                                 rhs=src[:, k:k + 2, sl],
                                                 start=(k == 0), stop=(k == KT - 2),
                                                 perf_mode=DRMODE)
                            # the fp8-range rescale rides the psum->sbuf copy
                            qraw = qrawp.tile([128, 512], BF, tag=f"qraw{m}",
                                                name=f"qraw{m}")[:, :ch]
                            nc.scalar.activation(out=qraw, in_=qp, func=AF.Copy,
                                                 scale=crs)
                            qps.append(qp); qraws.append(qraw)
                        for m in range(4):
                            rp = psat.tile([128, 512], F32, tag="op", name="op")[:, :ch]
                            nc.tensor.matmul(rp, lhsT=shiftm, rhs=qraws[m],
                                             start=True, stop=True)
                            # qraw·cos on Pool (all-SBUF), rp·sin on DVE
                            tcos = scratch.tile([128, 512], BF, tag="tcos", name="tcos")[:, :ch]
                            nc.gpsimd.tensor_tensor(out=tcos, in0=qraws[m],
                                                    in1=cos_t[:, :ch], op=ALU.mult)
                            tsin = scratch.tile([128, 512], BF, tag="tsin", name="tsin")[:, :ch]
                            nc.vector.tensor_tensor(out=tsin, in0=rp,
                                                    in1=sin_t[:, :ch], op=ALU.mult)
                            nc.gpsimd.tensor_tensor(out=dst[:, 4 * sg + m, sl],
                                                    in0=tcos, in1=tsin, op=ALU.add)

            # ---------- v projection (token-major, fp8 DoubleRow) ----------
            def v_project(dst, wdram, layer, src, n_src_tok, cross, vdesc):
                wsb = []
                for g in range(2):
                    wt = wst.tile([128, KT, 512], F8, tag="wstage", name="wstage")
                    nc.sync.dma_start(out=wt, in_=wdram[layer, g])
                    wsb.append(wt)
                for mt in range(n_src_tok // 128):
                    for g in range(2):
                        vp = psmm.tile([128, 512], F32, tag="mm", name="mm")
                        for k in range(0, KT, 2):
                            nc.tensor.matmul(vp, lhsT=src[:, k:k + 2, mt * 128:(mt + 1) * 128],
                                             rhs=wsb[g][:, k:k + 2, :],
                                             start=(k == 0), stop=(k == KT - 2),
                                             perf_mode=DRMODE)
                        if not cross:
                            if g == 0:
                                nc.scalar.activation(out=dst[:, mt, g * 512:(g + 1) * 512],
                                                     in_=vp, func=AF.Copy, scale=vdesc)
                            else:
                                nc.vector.tensor_scalar_mul(
                                    dst[:, mt, g * 512:(g + 1) * 512], vp, vdesc)
                        else:
                            nc.scalar.activation(out=dst[0:64, 2 * mt, g * 512:(g + 1) * 512],
                                                 in_=vp[0:64], func=AF.Copy, scale=vdesc)
                            nc.scalar.activation(out=dst[64:128, 2 * mt + 1, g * 512:(g + 1) * 512],
                                                 in_=vp[64:128], func=AF.Copy, scale=vdesc)

            # ---------- attention core ([k,q]-layout, fp8 e, PE denominators) ----------
            def attention(qT, kT, vv, o_all, n_k, cross, tail=None):
                # sim is computed transposed (out [k, q] per head) so the exp
                # output feeds attn@v directly; softmax denominators come from
                # a ones-matmul over the k partitions; the divide happens on
                # the [dh, q] attention output (16x fewer elements than e).
                order = [2 * i for i in range(HEADS // 2)] + [2 * i + 1 for i in range(HEADS // 2)]
                pos = {h: i for i, h in enumerate(order)}

                def phase_a(b):
                    base = 64 * (b % 2) if cross else 0
                    qsl = slice(b * 128, (b + 1) * 128)
                    ksl = slice(b * n_k, (b + 1) * n_k)
                    e_b = attn_pool.tile([128, HEADS * 128], F8, tag="e_b", name="e_b")
                    for g in range(4):  # 4 heads per psum tile
                        heads = order[g * 4:(g + 1) * 4]
                        sp = psmm.tile([128, 512], F32, tag="mm", name="mm")
                        for j, h in enumerate(heads):
                            hb = 64 * (h % 2)
                            nc.tensor.matmul(
                                sp[base:base + n_k, j * 128:(j + 1) * 128],
                                lhsT=kT[hb:hb + 64, h // 2, ksl],
                                rhs=qT[hb:hb + 64, h // 2, qsl],
                                start=(j == 0), stop=(j == 3),
                                skip_group_check=True)
                        nc.scalar.activation(out=e_b[base:base + n_k, g * 512:(g + 1) * 512],
                                             in_=sp[base:base + n_k], func=AF.Exp,
                                             scale=exp_s)
                    return e_b

                def phase_b(b, e_b):
                    base = 64 * (b % 2) if cross else 0
                    qsl = slice(b * 128, (b + 1) * 128)
                    for half in range(2):
                        # denominators for this half's 8 heads, broadcast to all
                        # 64 partitions by an all-ones stationary: evens on top
                        # partitions, odds on bottom (matches op_ layout)
                        dD = psat.tile([128, 512], F32, tag="dD", name="dD")
                        for par in range(2):
                            esl = slice(par * 1024 + half * 512,
                                        par * 1024 + half * 512 + 512)
                            nc.tensor.matmul(dD[64 * par:64 * par + 64],
                                             lhsT=ones_mat[base:base + n_k, :64],
                                             rhs=e_b[base:base + n_k, esl],
                                             start=True, stop=True,
                                             skip_group_check=True)
                        dr = attn_pool.tile([128, 512], BF, tag="dr", name="dr")
                        with nc.allow_low_precision(reason="bf16 softmax denom"):
                            nc.vector.reciprocal(out=dr, in_=dD)
                        op_ = psat.tile([128, 512], F32, tag="op", name="op")
                        for hp in range(4):
                            h0 = 8 * half + 2 * hp
                            nc.tensor.matmul(
                                op_[0:64, hp * 128:(hp + 1) * 128],
                                lhsT=vv[base:base + n_k, b, h0 * 64:(h0 + 1) * 64],
                                rhs=e_b[base:base + n_k, pos[h0] * 128:(pos[h0] + 1) * 128],
                                start=(hp == 0), stop=False, tile_position=(base, 0),
                                skip_group_check=True)
                            nc.tensor.matmul(
                                op_[64:128, hp * 128:(hp + 1) * 128],
                                lhsT=vv[base:base + n_k, b, (h0 + 1) * 64:(h0 + 2) * 64],
                                rhs=e_b[base:base + n_k, pos[h0 + 1] * 128:(pos[h0 + 1] + 1) * 128],
                                start=(hp == 0), stop=(hp == 3), tile_position=(base, 64),
                                skip_group_check=True)
                        nc.vector.tensor_tensor(
                            out=o_all[:, 4 * half:4 * half + 4, qsl],
                            in0=op_.rearrange("p (hp t) -> p hp t", hp=4),
                            in1=dr.rearrange("p (hp t) -> p hp t", hp=4),
                            op=ALU.mult)

                prev = None
                for b in range(BB):
                    e_b = phase_a(b)
                    if prev is not None:
                        phase_b(prev[0], prev[1])
                        # once batches 0..3 (= token chunk 0) are done, spread
                        # the chunk-0 out-projection pieces over the remaining
                        # batch iterations to fill PE bubbles
                        if prev[0] >= 3 and tail is not None:
                            tail(0, prev[0] - 3)
                    prev = (b, e_b)
                phase_b(prev[0], prev[1])
                if tail is not None:
                    for piece in range(4):
                        tail(1, piece)

            # ---------- output projection + residual (fp8 DoubleRow) ----------
            def out_project_staged(wdram, layer):
                wsb = []
                for g in range(2):
                    wt = wst.tile([128, KT, 512], F8, tag="wstage", name="wstage")
                    nc.sync.dma_start(out=wt, in_=wdram[layer, g])
                    wsb.append(wt)
                return wsb

            def out_project_chunk(wsb, src, odesc, c, piece):
                sl = slice(c * CH, (c + 1) * CH)
                for m in range(2 * piece, 2 * piece + 2):
                    pp = psmm.tile([128, 512], F32, tag="mm", name="mm")[:, :CH]
                    for k in range(0, KT, 2):
                        nc.tensor.matmul(pp,
                                         lhsT=wsb[m // 4][:, k:k + 2, (m % 4) * 128:(m % 4 + 1) * 128],
                                         rhs=src[:, k:k + 2, sl],
                                         start=(k == 0), stop=(k == KT - 2),
                                         perf_mode=DRMODE)
                    nc.vector.scalar_tensor_tensor(out=xT[:, m, sl], in0=pp,
                                                   scalar=odesc, in1=xT[:, m, sl],
                                                   op0=ALU.mult, op1=ALU.add)

            # ---------- FFN (weights staged once; hT holds both chunks) ----------
            def ffn(layer, xn):
                gelu_f = AF.Gelu if gelu_exact else AF.Square
                hT = big.tile([128, FF // 128, TOK], BF, tag="obuf", name="obuf")
                for g in range(FF // 512):
                    wt = wst.tile([128, KT, 512], BF, tag="wstage", name="wstage")
                    nc.sync.dma_start(out=wt, in_=w1[layer, g])
                    for c in range(NCH):
                        sl = slice(c * CH, (c + 1) * CH)
                        for mm in range(4):
                            fm = 4 * g + mm
                            hp = psmm.tile([128, 512], F32, tag="mm", name="mm")[:, :CH]
                            for k in range(KT):
                                nc.tensor.matmul(hp, lhsT=wt[:, k, mm * 128:(mm + 1) * 128],
                                                 rhs=xn[:, k, sl],
                                                 start=(k == 0), stop=(k == KT - 1))
                            nc.scalar.activation(out=hT[:, fm, sl], in_=hp, func=gelu_f,
                                                 bias=b1_sb[:, layer, fm, None], scale=1.0)
                for qm in range(4):  # 256-wide output blocks, both chunks at once
                    yps = [psmm.tile([128, 512], F32, tag="mm", name="mm")[:, :CH]
                           for _ in range(4)]  # index 2*c + mm
                    for kg in range(4):
                        wt = wst.tile([128, KT, 256], BF, tag="wstage2", name="wstage2")
                        nc.sync.dma_start(out=wt, in_=w2[layer, kg, qm])
                        for c in range(NCH):
                            sl = slice(c * CH, (c + 1) * CH)
                            for k in range(KT):
                                for mm in range(2):
                                    nc.tensor.matmul(
                                        yps[2 * c + mm],
                                        lhsT=wt[:, k, mm * 128:(mm + 1) * 128],
                                        rhs=hT[:, kg * KT + k, sl],
                                        start=(kg == 0 and k == 0),
                                        stop=(kg == 3 and k == KT - 1))
                    for c in range(NCH):
                        sl = slice(c * CH, (c + 1) * CH)
                        for mm in range(2):
                            nc.vector.tensor_tensor(out=xT[:, 2 * qm + mm, sl],
                                                    in0=yps[2 * c + mm],
                                                    in1=xT[:, 2 * qm + mm, sl],
                                                    op=ALU.add)

            # ================= main =================
            for layer in range(DEPTH):
                xn = big.tile([128, KT, TOK], F8, tag="xn", name="xn")
                rmsnorm(xn, sx=SXA)
                qT = big.tile([128, KT, TOK], F8, tag="qbuf", name="qbuf")
                kT = big.tile([128, KT, TOK], F8, tag="kbuf", name="kbuf")
                vv = big.tile([128, BB, INNER], F8, tag="vbuf", name="vbuf")
                o_all = big.tile([128, KT, TOK], F8, tag="obuf", name="obuf")
                qk_project(qT, wq, layer, xn, TOK, cosq, sinq, crs=cq_a)
                qk_project(kT, wk, layer, xn, TOK, cosq, sinq, crs=ck_a)
                v_project(vv, wv, layer, xn, TOK, cross=False, vdesc=vdesc_a)
                wsb_o = out_project_staged(wo, layer)
                attention(qT, kT, vv, o_all, 128, cross=False,
                          tail=lambda c, p: out_project_chunk(wsb_o, o_all, odesc_a, c, p))

                xn = big.tile([128, KT, TOK], F8, tag="xn", name="xn")
                rmsnorm(xn, sx=SXA)
                qT = big.tile([128, KT, TOK], F8, tag="qbuf", name="qbuf")
                kTc = big.tile([128, KT, TOKC], F8, tag="kbuf", name="kbuf")
                vvc = big.tile([128, BB, INNER], F8, tag="vbuf", name="vbuf")
                o_all = big.tile([128, KT, TOK], F8, tag="obuf", name="obuf")
                qk_project(qT, wqc, layer, xn, TOK, cosq, sinq, crs=cq_c)
                qk_project(kTc, wkc, layer, csT, TOKC, cosc, sinc, crs=ck_c)
                v_project(vvc, wvc, layer, csT, TOKC, cross=True, vdesc=vdesc_c)
                wsb_oc = out_project_staged(woc, layer)
                attention(qT, kTc, vvc, o_all, M_CTX, cross=True,
                          tail=lambda c, p: out_project_chunk(wsb_oc, o_all, odesc_c, c, p))

                xn = big.tile([128, KT, TOK], BF, tag="xn", name="xn")
                rmsnorm(xn)
                ffn(layer, xn)

            rmsnorm(None, dst_f32=outT, gcol=None if gout_ones else gout_sb)

    nc.compile()
    return nc


_NC_CACHE = {}


def _get_nc(BB, DEPTH, scales, gelu_exact=True, has_bias=False):
    key = (BB, DEPTH, scales, gelu_exact, has_bias)
    if key not in _NC_CACHE:
        _NC_CACHE[key] = _build(BB, DEPTH, scales, gelu_exact, has_bias)
    return _NC_CACHE[key]


def _fp8_scale(w):
    """Largest power-of-2 scale keeping |w*s| <= 192 (e4m3 max finite 240)."""
    mx = float(np.abs(w).max())
    return float(2.0 ** np.floor(np.log2(192.0 / max(mx, 1e-30))))


def _stage2d(W, nblk):
    """[L, K, M] -> [L, nblk, 128, K//128, 512] staging layout (contiguous per
    partition per block, so each staging DMA is 128 linear descriptors)."""
    L, K, M = W.shape
    kt = K // 128
    assert M == nblk * 512
    return np.ascontiguousarray(
        W.reshape(L, kt, 128, nblk, 512).transpose(0, 3, 2, 1, 4))


def _stage_w2(W2):
    """[L, FF, DIM] -> [L, 4 kg, 4 qm, 128, KT, 256]."""
    L = W2.shape[0]
    return np.ascontiguousarray(
        W2.reshape(L, 4, KT, 128, 4, 256).transpose(0, 1, 4, 3, 2, 5))


def _to_fp8(w, s):
    q = (np.asarray(w, np.float32) * s).astype(E4)
    assert np.isfinite(q.astype(np.float32)).all()
    return q


def _rope_tables(n_pos, n_cols):
    """Masked full-head tables [128, n_cols]: rope rows (d%64<32) carry cos/sin,
    pass rows carry cos=1, sin=0. Token columns are batch-periodic."""
    inv = 1.0 / (10000.0 ** (np.arange(0, ROT, 2, dtype=np.float32) / ROT))  # [16]
    pos = np.arange(n_cols, dtype=np.float32) % n_pos
    d = np.arange(64)
    f = inv[d % 16]
    ang = f[:, None] * pos[None, :]
    cos = np.cos(ang)
    sin = np.sin(ang) * np.where(d % 32 < 16, -1.0, 1.0).astype(np.float32)[:, None]
    mask_rope = (d < 32)[:, None]
    cos = np.where(mask_rope, cos, 1.0)
    sin = np.where(mask_rope, sin, 0.0)
    return (np.tile(cos, (2, 1)).astype(BF16), np.tile(sin, (2, 1)).astype(BF16))


def _pack_qk(W):
    return W  # natural layout; rotation happens on-device via the shift matmul


def _shift_matrix():
    """S [128,128] bf16: out[m] = in[src(m)] for rope rows, 0 for pass rows."""
    S = np.zeros((128, 128), np.float32)
    for m in range(128):
        d = m % 64
        if d < 32:
            S[64 * (m // 64) + (d + 16) % 32, m] = 1.0
    return S.astype(BF16)


def _prep_weights(inputs, DEPTH):
    f32 = np.float32
    g_attn = np.asarray(inputs["g_attn"], f32)
    g_cross = np.asarray(inputs["g_cross"], f32)
    g_ff = np.asarray(inputs["g_ff"], f32)
    out = {}
    wq_l, wk_l, wv_l, wqc_l, wkc_l, wvc_l = [], [], [], [], [], []
    for L in range(DEPTH):
        Wq = np.asarray(inputs["Wq_a"][L], f32) * g_attn[L][:, None] * (DH ** -0.5)
        Wkv = np.asarray(inputs["Wkv_a"][L], f32) * g_attn[L][:, None]
        wq_l.append(_pack_qk(Wq))
        wk_l.append(_pack_qk(Wkv[:, :INNER]))
        wv_l.append(Wkv[:, INNER:])
        Wqc = np.asarray(inputs["Wq_c"][L], f32) * g_cross[L][:, None] * (DH ** -0.5)
        Wkvc = np.asarray(inputs["Wkv_c"][L], f32)  # context is not normed
        wqc_l.append(_pack_qk(Wqc))
        wkc_l.append(_pack_qk(Wkvc[:, :INNER]))
        wvc_l.append(Wkvc[:, INNER:])
    wo_f = np.asarray(inputs["Wo_a"], f32)[:DEPTH]
    woc_f = np.asarray(inputs["Wo_c"], f32)[:DEPTH]
    stacks = dict(wq=np.stack(wq_l), wk=np.stack(wk_l), wv=np.stack(wv_l),
                  wo=wo_f, wqc=np.stack(wqc_l), wkc=np.stack(wkc_l),
                  wvc=np.stack(wvc_l), woc=woc_f)
    scales = tuple(_fp8_scale(stacks[n]) for n in
                   ("wq", "wk", "wv", "wo", "wqc", "wkc", "wvc", "woc"))
    scales = scales + (bool(np.all(np.asarray(inputs["g_out"], f32) == 1.0)),)
    for n, s in zip(("wq", "wk", "wv", "wo", "wqc", "wkc", "wvc", "woc"), scales[:8]):
        out[n] = _stage2d(_to_fp8(stacks[n], s), 2)
    out["w1"] = _stage2d(
        (np.asarray(inputs["W1"], f32)[:DEPTH] * g_ff[:DEPTH, :, None]).astype(BF16), 8)
    out["w2"] = _stage_w2(np.asarray(inputs["W2"], f32)[:DEPTH].astype(BF16))
    out["b1c"] = np.ascontiguousarray(
        np.asarray(inputs["b1"], f32)[:DEPTH].reshape(DEPTH, FF // 128, 128).transpose(0, 2, 1))
    out["goutc"] = np.ascontiguousarray(np.asarray(inputs["g_out"], f32).reshape(KT, 128).T)
    brows = np.stack([np.asarray(inputs["bo_a"], f32)[:DEPTH],
                      np.asarray(inputs["bo_c"], f32)[:DEPTH],
                      np.asarray(inputs["b2"], f32)[:DEPTH]], axis=1)
    has_bias = bool(np.any(brows))
    assert not has_bias, "fp8 out_project path dropped the bias matmul"
    return out, has_bias, scales


def prepare(inputs, BB, DEPTH, n_cores):
    """Returns (in_maps, has_bias, scales) for n_cores cores."""
    TOK, TOKC = BB * N_CTX, BB * M_CTX
    CH, CHC = min(512, TOK), min(512, TOKC)
    w, has_bias, scales = _prep_weights(inputs, DEPTH)
    cosq, sinq = _rope_tables(N_CTX, CH)
    cosc, sinc = _rope_tables(M_CTX, CHC)
    x = np.asarray(inputs["x"], np.float32)
    cs = np.asarray(inputs["chunked_seq"], np.float32)
    in_maps = []
    for c in range(n_cores):
        xs = x[c * BB:(c + 1) * BB]
        css = cs[c * BB:(c + 1) * BB]
        m = dict(w)
        m["xT"] = np.ascontiguousarray(xs.reshape(TOK, DIM).T)
        m["csT"] = _to_fp8(np.ascontiguousarray(css.reshape(TOKC, DIM).T), SXC)
        m["cosq"], m["sinq"] = cosq, sinq
        m["shiftm"] = _shift_matrix()
        m["cosc"], m["sinc"] = cosc, sinc
        in_maps.append(m)
    return in_maps, has_bias, scales


def run_cores(inputs, BB, DEPTH, n_cores, gelu_exact=True):
    in_maps, has_bias, scales = prepare(inputs, BB, DEPTH, n_cores)
    nc = _get_nc(BB, DEPTH, scales, gelu_exact, has_bias)
    res = run_bass_kernel_spmd(nc, in_maps, list(range(n_cores)))
    outs = []
    for c in range(n_cores):
        oT = res.results[c]["outT"]
        outs.append(np.asarray(oT, np.float32).T.reshape(BB, N_CTX, DIM))
    return np.concatenate(outs, axis=0)


def kernel(**inputs):
    return run_cores(inputs, BB=8, DEPTH=3, n_cores=N_CORES).astype(np.float32)

